# revision 21
# baseline (speedup 1.0000x reference)
"""DiffSeg segmentation head on 8 Trainium2 NeuronCores (Bass/Tile).

ONE bass program (l123), ONE dispatch, ONE blocking host fetch per image:
  L1 section: multi-scale aggregation of attention maps -> agg [4096,4096]
      f32 (row-sharded 512 rows/core), anchor gather + symmetric-KL merge0
      (fp16 log/matmuls like the reference), ReduceScatter -> X2 p-slices,
      C1 = X2 @ lX2.T + sx packed into one [257,256] AllReduce.
  greedy 2 (on-device, replicated per core): klmat assembled from the
      AllReduced pack with the reference's exact f32 op order; the
      sequential 256-step greedy selection runs as an unrolled loop
      (PE one-hot row extraction + 3 DVE ops per step) in SCATTERED form:
      rows stay at their anchor index, an active mask a2[] marks
      representatives (compaction is order-preserving so all downstream
      values are identical).
  L2 section: selT = adjacency*mask, new2 = sel @ X2 / cnt (PE), AllGather,
      window gather, masked ln, C3/sx3 pack -> [257,256] AllReduce.
  greedy 3: same unrolled loop with valid = a2 -> a3.
  L3 section: new3 = sel3 @ new2_window / cnt + (-1e38 inactive bias),
      4x bilinear upsample (align_corners), transpose, per-pixel argmax,
      labels packed as u8 together with a3, AllGather -> single output.
  host: one fetch of the [8,128,66] u8 pack; scattered argmax indices are
      remapped to the reference's compacted labels via a cumsum LUT on a3.

Every host round-trip over the axon tunnel costs ~30-90ms (ambient), so
the whole computation is a single async dispatch + a single fetch; the
on-device pipeline itself adds only ~4ms.

Self-contained: hardcodes shapes/sharding for inputs
  weight_64 [B,8,4096,4096], weight_32 [B,8,1024,1024],
  weight_16 [B,8,256,256],   weight_8  [B,8,64,64]  (B=1)
"""

import sys
import numpy as np

for _p in ("/opt/trn_rl_repo", "/opt/trn_rl_repo/concourse"):
    if _p not in sys.path:
        sys.path.append(_p)

NCORES = 8
THR2 = np.float32(1.8)          # == 2 * f32(0.9), exact
RAT = [np.float32(64.0 / 120.0), np.float32(32.0 / 120.0),
       np.float32(16.0 / 120.0), np.float32(8.0 / 120.0)]
NEG_BIG = np.float32(-1e38)


def _up_coords(n, r):
    s = np.linspace(0.0, n - 1.0, n * r)
    i0 = np.clip(np.floor(s).astype(np.int64), 0, n - 2)
    w = (s - i0).astype(np.float32)
    return i0.astype(np.int64), w


# ---------------------------------------------------------------- host consts
X032, W32 = _up_coords(32, 2)      # 32 -> 64
X016, W16 = _up_coords(16, 4)
X08, W8 = _up_coords(8, 8)
Y0U, WYU = _up_coords(64, 4)       # 64 -> 256 (final upsample)
X0U, WXU = _up_coords(64, 4)

# L3 per-core y windows
L3_LO = [int(Y0U[32 * k]) for k in range(NCORES)]          # window start row
L3_W = 12  # padded window size (12 = smallest mult-of-4 >= 10)

ANCHOR_PTS = [1 + 4 * i for i in range(16)]


def _wrap_idx(vals):
    """ap_gather index layout: j -> partition j%16, elem j//16; replicated to
    all 8 gpsimd cores (16-partition groups)."""
    n = len(vals)
    cols = (n + 15) // 16
    arr = np.zeros((16, cols), np.int16)
    for j, v in enumerate(vals):
        arr[j % 16, j // 16] = v
    return np.tile(arr, (8, 1))


def _l3_host_consts():
    consts = []
    # x-gather indices (uniform across cores): view [32y*64x]; idx = y*64+x0(j)
    idxx_c = np.array([y * 64 + X0U[j] for y in range(32) for j in range(256)],
                      np.int16)
    idxx_d = np.array([y * 64 + X0U[j] + 1 for y in range(32) for j in range(256)],
                      np.int16)
    wxr = np.broadcast_to(WXU[None, :], (32, 256)).reshape(1, -1).astype(np.float32)
    for k in range(NCORES):
        lo = L3_LO[k]
        y0l = [int(Y0U[32 * k + t]) - lo for t in range(32)]
        consts.append(dict(
            idxyc=_wrap_idx(np.array(y0l, np.int16)),
            idxyd=_wrap_idx(np.array(y0l, np.int16) + 1),
            wy=WYU[32 * k:32 * k + 32][None, :].astype(np.float32),
            idxxc=_wrap_idx(idxx_c),
            idxxd=_wrap_idx(idxx_d),
            wx=WXU[None, :].astype(np.float32),
        ))
    return consts


L3C = _l3_host_consts()
_IWIN_CAT = np.concatenate(
    [_wrap_idx(np.clip(np.arange(L3_LO[k], L3_LO[k] + L3_W), 0, 63)
               .astype(np.int16)) for k in range(NCORES)], axis=0)
_L3_DEV = {}


def _l3_const_dev():
    """Constant L3 inputs, concatenated and staged on device once."""
    if not _L3_DEV:
        import jax
        from jax.sharding import Mesh, PartitionSpec, NamedSharding
        mesh = Mesh(np.asarray(jax.devices()[:NCORES]), ("core",))
        sh = NamedSharding(mesh, PartitionSpec("core"))
        for nm in ("wy", "wx", "idxyc", "idxyd", "idxxc", "idxxd"):
            cat = np.concatenate([L3C[k][nm] for k in range(NCORES)], axis=0)
            _L3_DEV[nm] = jax.device_put(cat, sh)
    return _L3_DEV

# ------------------------------------------------------------------- programs
_PROGS = {}


def _mybir():
    from concourse import mybir
    return mybir


def _build_common():
    import concourse.tile as tile
    from concourse import mybir, bacc
    from concourse.bass_utils import axon_active
    nc = bacc.Bacc("TRN2", target_bir_lowering=False, debug=False,
                   enable_asserts=False, num_devices=NCORES)
    return nc, tile, mybir



def _emit_l1(nc, tile, mybir, tc, w64, w32, w16, w8, o_x2, o_kl):
    from concourse.masks import make_identity
    F32, F16 = mybir.dt.float32, mybir.dt.float16
    ALU, AX, AF = mybir.AluOpType, mybir.AxisListType, mybir.ActivationFunctionType
    RG = [list(range(NCORES))]
    if True:
        with tc.tile_pool(name="consts", bufs=1) as cpool, \
             tc.tile_pool(name="ystore", bufs=1) as ystore, \
             tc.tile_pool(name="dram", bufs=1, space="DRAM") as dram:

            ident16 = cpool.tile([128, 128], F16, tag="id16", name="id16")
            make_identity(nc, ident16[:])
            ident32 = cpool.tile([128, 128], F32, tag="id32", name="id32")
            make_identity(nc, ident32[:])
            ones16 = cpool.tile([128, 1], F16, tag="o16", name="o16")
            nc.gpsimd.memset(ones16[:], 1.0)
            ones32 = cpool.tile([128, 1], F32, tag="o32", name="o32")
            nc.gpsimd.memset(ones32[:], 1.0)

            Y = [ystore.tile([128, 4096], F32, tag=f"Y{pt}", name=f"Y{pt}")
                 for pt in range(4)]

            # ============ phases A+B: aggregation ============
            with tc.tile_pool(name="nmap", bufs=1) as nmap, \
                 tc.tile_pool(name="stage", bufs=2) as stage, \
                 tc.tile_pool(name="workab", bufs=1) as workab:

                def upsample_norm(src_dram, s, P, tidx, out_tile):
                    """head-sum -> bilinear s->64 (x then y) -> normalize."""
                    ssq = s * s
                    m = workab.tile([P, ssq], F32, tag="mA", name="mA")
                    ncb = max(1, ssq // 256)
                    cw = ssq // ncb
                    for cb in range(ncb):
                        stg = stage.tile([P, 8, cw], F32, tag="stg", name="stg")
                        for h in range(8):
                            nc.sync.dma_start(
                                stg[:, h, :],
                                src_dram[h, tidx * P:(tidx + 1) * P,
                                         cb * cw:(cb + 1) * cw])
                        nc.vector.tensor_reduce(
                            m[:, cb * cw:(cb + 1) * cw],
                            stg[:].rearrange("p h x -> p x h"), AX.X, ALU.add)
                    m3 = m[:].rearrange("p (y x) -> p y x", y=s)
                    x0s, wxs = {32: (X032, W32), 16: (X016, W16),
                                8: (X08, W8)}[s]
                    ux = workab.tile([P, s, 64], F32, tag="uxA", name="uxA")
                    tmp = workab.tile([P, s], F32, tag="tmpxA", name="tmpxA")
                    for j in range(64):
                        c = m3[:, :, int(x0s[j])]
                        d = m3[:, :, int(x0s[j]) + 1]
                        eng = nc.vector if j % 2 == 0 else nc.gpsimd
                        eng.tensor_tensor(tmp[:], d, c, ALU.subtract)
                        nc.vector.scalar_tensor_tensor(
                            ux[:, :, j], tmp[:], float(wxs[j]), c,
                            ALU.mult, ALU.add)
                    tmp2 = workab.tile([P, 64], F32, tag="tmpyA", name="tmpyA")
                    for j in range(64):
                        c = ux[:, int(x0s[j]), :]
                        d = ux[:, int(x0s[j]) + 1, :]
                        eng = nc.vector if j % 2 == 0 else nc.gpsimd
                        eng.tensor_tensor(tmp2[:], d, c, ALU.subtract)
                        nc.vector.scalar_tensor_tensor(
                            out_tile[:, j, :], tmp2[:], float(wxs[j]), c,
                            ALU.mult, ALU.add)
                    rs = workab.tile([P, 1], F32, tag="rsA", name="rsA")
                    nc.vector.tensor_reduce(rs[:], out_tile[:], AX.XY, ALU.add)
                    nc.vector.reciprocal(rs[:], rs[:])
                    flat = out_tile[:].rearrange("p a b -> p (a b)")
                    nc.vector.tensor_scalar(flat, flat, rs[:], None, ALU.mult)

                n32 = [nmap.tile([128, 64, 64], F32, tag=f"n32_{t}",
                                 name=f"n32_{t}") for t in range(2)]
                for t in range(2):
                    upsample_norm(w32, 32, 128, t, n32[t])
                n16 = nmap.tile([128, 64, 64], F32, tag="n16", name="n16")
                upsample_norm(w16, 16, 128, 0, n16)
                n8 = nmap.tile([64, 64, 64], F32, tag="n8", name="n8")
                upsample_norm(w8, 8, 64, 0, n8)

                for pt in range(4):
                    for cb in range(8):
                        stg = stage.tile([128, 8, 512], F32, tag="stg",
                                         name="stg64")
                        for h in range(8):
                            nc.sync.dma_start(
                                stg[:, h, :],
                                w64[h, pt * 128:(pt + 1) * 128,
                                    cb * 512:(cb + 1) * 512])
                        nc.vector.tensor_reduce(
                            Y[pt][:, cb * 512:(cb + 1) * 512],
                            stg[:].rearrange("p h x -> p x h"), AX.X, ALU.add)
                    rs = workab.tile([128, 1], F32, tag="rsY", name="rsY")
                    nc.vector.tensor_reduce(rs[:], Y[pt][:], AX.X, ALU.add)
                    nc.vector.reciprocal(rs[:], rs[:])
                    nc.vector.tensor_scalar(rs[:], rs[:], float(RAT[0]), None,
                                            ALU.mult)
                    nc.vector.tensor_scalar(Y[pt][:], Y[pt][:], rs[:], None,
                                            ALU.mult)
                    rep = workab.tile([128, 4096], F32, tag="rep", name="rep")
                    srct = n32[pt // 2]
                    base = (2 * pt) % 4 * 32
                    for ar in range(2):
                        for rp in range(2):
                            nc.sync.dma_start(
                                rep[ar * 64 + rp * 32:ar * 64 + rp * 32 + 32, :],
                                srct[base + ar * 32:base + ar * 32 + 32, :, :]
                                .rearrange("p a b -> p (a b)"))
                    nc.vector.scalar_tensor_tensor(
                        Y[pt][:], rep[:], float(RAT[1]), Y[pt][:],
                        ALU.mult, ALU.add)
                    rep2 = workab.tile([128, 4096], F32, tag="rep", name="rep2")
                    for ar in range(2):
                        a_loc = 2 * pt + ar
                        for rp in range(4):
                            nc.sync.dma_start(
                                rep2[ar * 64 + rp * 16:ar * 64 + rp * 16 + 16, :],
                                n16[a_loc * 16:a_loc * 16 + 16, :, :]
                                .rearrange("p a b -> p (a b)"))
                    nc.vector.scalar_tensor_tensor(
                        Y[pt][:], rep2[:], float(RAT[2]), Y[pt][:],
                        ALU.mult, ALU.add)
                    rep3 = workab.tile([128, 4096], F32, tag="rep", name="rep3")
                    for ar in range(2):
                        a_loc = 2 * pt + ar
                        for rp in range(8):
                            nc.sync.dma_start(
                                rep3[ar * 64 + rp * 8:ar * 64 + rp * 8 + 8, :],
                                n8[(a_loc % 8) * 8:(a_loc % 8) * 8 + 8, :, :]
                                .rearrange("p a b -> p (a b)"))
                    nc.vector.scalar_tensor_tensor(
                        Y[pt][:], rep3[:], float(RAT[3]), Y[pt][:],
                        ALU.mult, ALU.add)

            # ============ phase C: merge0 ============
            with tc.tile_pool(name="xstore", bufs=1) as xstore, \
                 tc.tile_pool(name="workc", bufs=2) as workc:
                # natural-layout fp16 + sy columns
                yh_nat = [xstore.tile([128, 4096], F16, tag=f"yhn{pt}",
                                      name=f"yhn{pt}") for pt in range(4)]
                sycol = [cpool.tile([128, 1], F32, tag=f"syc{mt}",
                                    name=f"syc{mt}") for mt in range(4)]
                for pt in range(4):
                    nc.scalar.activation(yh_nat[pt][:], Y[pt][:], AF.Copy)
                    lnn = workc.tile([128, 4096], F16, tag="lnn", name="lnn",
                                     bufs=1)
                    nc.scalar.activation(lnn[:], yh_nat[pt][:], AF.Ln)
                    nc.vector.tensor_tensor(lnn[:], yh_nat[pt][:], lnn[:],
                                            ALU.mult)
                    nc.vector.tensor_reduce(sycol[pt][:], lnn[:], AX.X,
                                            ALU.add)

                # anchors -> allgather -> X^T, lX^T
                psA_cm = tc.tile_pool(name="psA", bufs=2, space="PSUM")
                psA = psA_cm.__enter__()
                psAs_cm = tc.tile_pool(name="psAs", bufs=1, space="PSUM")
                psAs = psAs_cm.__enter__()
                xloc = workc.tile([32, 4096], F32, tag="xloc", name="xloc",
                                  bufs=1)
                nc.sync.dma_start(xloc[0:16, :], Y[0][65:126:4, :])
                nc.sync.dma_start(xloc[16:32, :], Y[2][65:126:4, :])
                xloch = workc.tile([32, 4096], F16, tag="xloch", name="xloch",
                                   bufs=1)
                nc.scalar.activation(xloch[:], xloc[:], AF.Copy)
                bx_in = dram.tile([32, 4096], F16, name="bx_in")
                bx_out = dram.tile([8, 32, 4096], F16, name="bx_out")
                nc.sync.dma_start(bx_in[:], xloch[:])
                nc.gpsimd.collective_compute(
                    "AllGather", ALU.bypass, replica_groups=RG,
                    ins=[bx_in.opt()], outs=[bx_out.opt()])
                xT = xstore.tile([128, 32, 256], F16, tag="xT", name="xT")
                lxT = xstore.tile([128, 32, 256], F16, tag="lxT", name="lxT")
                bxv = bx_out[:].rearrange("c a p -> (c a) p")
                for nt in range(2):
                    xnat = workc.tile([128, 4096], F16, tag="xnat",
                                      name="xnat", bufs=1)
                    nc.sync.dma_start(xnat[:], bxv[nt * 128:(nt + 1) * 128, :])
                    for ct in range(32):
                        pst = psA.tile([128, 128], F16, tag="tp16",
                                       name="tp16")
                        nc.tensor.transpose(
                            pst[:], xnat[:, ct * 128:(ct + 1) * 128],
                            ident16[:])
                        nc.any.tensor_copy(
                            xT[:, ct, nt * 128:(nt + 1) * 128], pst[:])
                for ct in range(32):
                    nc.scalar.activation(lxT[:, ct, :], xT[:, ct, :], AF.Ln)
                sxP = psAs.tile([1, 256], F32, tag="sx", name="sxP")
                for ct in range(32):
                    prodx = workc.tile([128, 256], F16, tag="prodX",
                                       name="prodX")
                    nc.vector.tensor_tensor(prodx[:], xT[:, ct, :],
                                            lxT[:, ct, :], ALU.mult)
                    nc.tensor.matmul(sxP[:], ones16[:], prodx[:],
                                     start=(ct == 0), stop=(ct == 31))
                sx_sb = workc.tile([1, 256], F32, tag="sxsb", name="sxsb",
                                   bufs=1)
                nc.any.tensor_copy(sx_sb[:], sxP[:])
                sxb = cpool.tile([128, 256], F32, tag="sxb", name="sxb")
                nc.gpsimd.partition_broadcast(sxb[:], sx_sb[:])

                # cross accumulation with rolling transposed ct-tiles
                psC = [psA.tile([128, 256], F32, tag=f"psC{mt}",
                                name=f"psC{mt}", bufs=1) for mt in range(4)]
                for ct in range(32):
                    yhTct = workc.tile([128, 512], F16, tag="yhTct",
                                       name="yhTct")
                    for pt in range(4):
                        pst = psA.tile([128, 128], F16, tag="tp16",
                                       name="tp16b")
                        nc.tensor.transpose(
                            pst[:], yh_nat[pt][:, ct * 128:(ct + 1) * 128],
                            ident16[:])
                        nc.any.tensor_copy(
                            yhTct[:, pt * 128:(pt + 1) * 128], pst[:])
                    lnct = workc.tile([128, 512], F16, tag="lnct", name="lnct")
                    nc.scalar.activation(lnct[:], yhTct[:], AF.Ln)
                    for mt in range(4):
                        nc.tensor.matmul(
                            psC[mt][:], lnct[:, mt * 128:(mt + 1) * 128],
                            xT[:, ct, :], start=(ct == 0), stop=False)
                        nc.tensor.matmul(
                            psC[mt][:], yhTct[:, mt * 128:(mt + 1) * 128],
                            lxT[:, ct, :], start=False, stop=(ct == 31))
                knT = xstore.tile([128, 4, 256], F32, tag="knT", name="knT")
                for mt in range(4):
                    S = workc.tile([128, 256], F32, tag="Ssum", name="Ssum")
                    nc.vector.tensor_scalar(S[:], sxb[:], sycol[mt][:], None,
                                            ALU.add)
                    nc.vector.tensor_tensor(S[:], S[:], psC[mt][:],
                                            ALU.subtract)
                    nc.vector.tensor_scalar(knT[:, mt, :], S[:], float(THR2),
                                            None, ALU.is_lt)

                psAs_cm.__exit__(None, None, None)
                psA_cm.__exit__(None, None, None)
                # new^T partials, counts, collectives
                psB_cm = tc.tile_pool(name="psB", bufs=2, space="PSUM")
                psB = psB_cm.__enter__()
                psBs_cm = tc.tile_pool(name="psBs", bufs=1, space="PSUM")
                psBs = psBs_cm.__enter__()
                cntP = psBs.tile([1, 256], F32, tag="cnt", name="cntP")
                for mt in range(4):
                    nc.tensor.matmul(cntP[:], ones32[:], knT[:, mt, :],
                                     start=(mt == 0), stop=(mt == 3))
                bN_in = dram.tile([4096, 256], F32, name="bN_in")
                for ptile in range(32):
                    psN = psB.tile([128, 256], F32, tag="psN", name="psN")
                    for mt in range(4):
                        nc.tensor.matmul(
                            psN[:], Y[mt][:, ptile * 128:(ptile + 1) * 128],
                            knT[:, mt, :], start=(mt == 0), stop=(mt == 3))
                    npt = workc.tile([128, 256], F32, tag="npt", name="npt")
                    nc.any.tensor_copy(npt[:], psN[:])
                    nc.sync.dma_start(bN_in[ptile * 128:(ptile + 1) * 128, :],
                                      npt[:])
                bC_in = dram.tile([1, 256], F32, name="bC_in")
                bC_out = dram.tile([1, 256], F32, name="bC_out")
                cnt_sb = workc.tile([1, 256], F32, tag="cntsb", name="cntsb",
                                    bufs=1)
                nc.any.tensor_copy(cnt_sb[:], cntP[:])
                nc.sync.dma_start(bC_in[:], cnt_sb[:])
                nc.gpsimd.collective_compute(
                    "AllReduce", ALU.add, replica_groups=RG,
                    ins=[bC_in.opt()], outs=[bC_out.opt()])
                bN_out = dram.tile([512, 256], F32, name="bN_out")
                nc.gpsimd.collective_compute(
                    "ReduceScatter", ALU.add, replica_groups=RG,
                    ins=[bN_in.opt()], outs=[bN_out.opt()])

                # X2^T, lX2^T, sx2 partial, C1 partial, outputs
                cntg = workc.tile([1, 256], F32, tag="cntg", name="cntg",
                                  bufs=1)
                nc.sync.dma_start(cntg[:], bC_out[:])
                nc.vector.reciprocal(cntg[:], cntg[:])
                cb2 = cpool.tile([128, 256], F32, tag="cb2", name="cb2")
                nc.gpsimd.partition_broadcast(cb2[:], cntg[:])
                x2T = [xstore.tile([128, 256], F32, tag=f"x2T{t}",
                                   name=f"x2T{t}") for t in range(4)]
                lx2T = [xstore.tile([128, 256], F32, tag=f"lx2T{t}",
                                    name=f"lx2T{t}") for t in range(4)]
                for t in range(4):
                    nc.sync.dma_start(x2T[t][:],
                                      bN_out[t * 128:(t + 1) * 128, :])
                    nc.vector.tensor_tensor(x2T[t][:], x2T[t][:], cb2[:],
                                            ALU.mult)
                    nc.scalar.activation(lx2T[t][:], x2T[t][:], AF.Ln)
                sx2P = psBs.tile([1, 256], F32, tag="sx2", name="sx2P")
                for t in range(4):
                    prod2 = workc.tile([128, 256], F32, tag="prod2",
                                       name="prod2")
                    nc.vector.tensor_tensor(prod2[:], x2T[t][:], lx2T[t][:],
                                            ALU.mult)
                    nc.tensor.matmul(sx2P[:], ones32[:], prod2[:],
                                     start=(t == 0), stop=(t == 3))
                sx2sb = workc.tile([1, 256], F32, tag="sx2sb", name="sx2sb",
                                   bufs=1)
                nc.any.tensor_copy(sx2sb[:], sx2P[:])
                bS_in = dram.tile([1, 256], F32, name="bS_in")
                bS_out = dram.tile([1, 256], F32, name="bS_out")
                nc.sync.dma_start(bS_in[:], sx2sb[:])
                nc.gpsimd.collective_compute(
                    "AllReduce", ALU.add, replica_groups=RG,
                    ins=[bS_in.opt()], outs=[bS_out.opt()])
                nc.sync.dma_start(o_kl[256:257, :], bS_out[:])
                bC1_in = dram.tile([256, 256], F32, name="bC1_in")
                bC1_out = dram.tile([256, 256], F32, name="bC1_out")
                for it in range(2):
                    psC1 = psB.tile([128, 256], F32, tag="pc1", name="pc1")
                    for kt in range(4):
                        nc.tensor.matmul(
                            psC1[:], x2T[kt][:, it * 128:(it + 1) * 128],
                            lx2T[kt][:], start=(kt == 0), stop=(kt == 3))
                    c1t = workc.tile([128, 256], F32, tag="c1t", name="c1t")
                    nc.any.tensor_copy(c1t[:], psC1[:])
                    nc.sync.dma_start(bC1_in[it * 128:(it + 1) * 128, :],
                                      c1t[:])
                nc.gpsimd.collective_compute(
                    "AllReduce", ALU.add, replica_groups=RG,
                    ins=[bC1_in.opt()], outs=[bC1_out.opt()])
                nc.sync.dma_start(o_kl[0:256, :], bC1_out[:])
                for it in range(2):
                    x2n = workc.tile([128, 512], F32, tag="x2n", name="x2n")
                    for kt in range(4):
                        pst = psB.tile([128, 128], F32, tag="tp32",
                                       name="tp32")
                        nc.tensor.transpose(
                            pst[:], x2T[kt][:, it * 128:(it + 1) * 128],
                            ident32[:])
                        nc.any.tensor_copy(x2n[:, kt * 128:(kt + 1) * 128],
                                           pst[:])
                    nc.sync.dma_start(o_x2[it * 128:(it + 1) * 128, :],
                                      x2n[:])
                psBs_cm.__exit__(None, None, None)
                psB_cm.__exit__(None, None, None)


def _build_l1():
    nc, tile, mybir = _build_common()
    F32 = mybir.dt.float32
    w64 = nc.dram_tensor("w64s", [8, 512, 4096], F32, kind="ExternalInput")
    w32 = nc.dram_tensor("w32s", [8, 256, 1024], F32, kind="ExternalInput")
    w16 = nc.dram_tensor("w16s", [8, 128, 256], F32, kind="ExternalInput")
    w8 = nc.dram_tensor("w8s", [8, 64, 64], F32, kind="ExternalInput")
    o_x2 = nc.dram_tensor("x2slice", [256, 512], F32, kind="ExternalOutput")
    o_kl = nc.dram_tensor("klpack", [257, 256], F32, kind="ExternalOutput")
    with tile.TileContext(nc) as tc:
        _emit_l1(nc, tile, mybir, tc, w64, w32, w16, w8, o_x2, o_kl)
    nc.finalize()
    return nc, ["w64s", "w32s", "w16s", "w8s"], ["x2slice", "klpack"]



def _build_l2():
    nc, tile, mybir = _build_common()
    from concourse.masks import make_identity
    F32 = mybir.dt.float32
    ALU, AX, AF = mybir.AluOpType, mybir.AxisListType, mybir.ActivationFunctionType

    x2s = nc.dram_tensor("x2s", [256, 512], F32, kind="ExternalInput")
    selT = nc.dram_tensor("sel2T", [256, 256], mybir.dt.uint8,
                          kind="ExternalInput")
    icnt = nc.dram_tensor("icnt2", [256, 1], F32, kind="ExternalInput")
    vrow = nc.dram_tensor("vrow", [1, 256], F32, kind="ExternalInput")
    irow = nc.dram_tensor("irow", [1, 256], F32, kind="ExternalInput")
    iwin = nc.dram_tensor("iwin", [128, 1], mybir.dt.int16,
                          kind="ExternalInput")
    o_n2w = nc.dram_tensor("n2w", [256, L3_W * 64], F32,
                           kind="ExternalOutput")
    o_kl3 = nc.dram_tensor("klpack3", [257, 256], F32, kind="ExternalOutput")
    RG = [list(range(NCORES))]

    with tile.TileContext(nc) as tc:
        with tc.tile_pool(name="sb", bufs=1) as pool, \
             tc.tile_pool(name="work", bufs=2) as work, \
             tc.tile_pool(name="psum", bufs=2, space="PSUM") as psum, \
             tc.tile_pool(name="psumS", bufs=1, space="PSUM") as psumS, \
             tc.tile_pool(name="dram", bufs=1, space="DRAM") as dram:
            ident32 = pool.tile([128, 128], F32, tag="id32", name="id32")
            make_identity(nc, ident32[:])
            ones32 = pool.tile([128, 1], F32, tag="o32", name="o32")
            nc.gpsimd.memset(ones32[:], 1.0)
            iw = pool.tile([128, 1], mybir.dt.int16, tag="iw", name="iw")
            nc.sync.dma_start(iw[:], iwin[:])
            xs = [pool.tile([128, 512], F32, tag=f"xs{t}", name=f"xs{t}") for t in range(2)]
            st = [pool.tile([128, 256], F32, tag=f"st{t}", name=f"st{t}") for t in range(2)]
            stu = [pool.tile([128, 256], mybir.dt.uint8, tag=f"stu{t}",
                             name=f"stu{t}") for t in range(2)]
            for t in range(2):
                nc.sync.dma_start(xs[t][:], x2s[t * 128:(t + 1) * 128, :])
                nc.sync.dma_start(stu[t][:], selT[t * 128:(t + 1) * 128, :])
                nc.any.tensor_copy(st[t][:], stu[t][:])
            cnt = pool.tile([128, 2], F32, tag="cnt", name="cnt")
            nc.sync.dma_start(cnt[:], icnt[:].rearrange("(a p) b -> p (a b)", a=2))
            rc = pool.tile([128, 2], F32, tag="rc", name="rc")
            nc.vector.reciprocal(rc[:], cnt[:])
            vb = pool.tile([128, 256], F32, tag="vb", name="vb")
            ib = pool.tile([128, 256], F32, tag="ib", name="ib")
            vsb = work.tile([1, 256], F32, tag="vsb", name="vsb")
            isb = work.tile([1, 256], F32, tag="isb", name="isb")
            nc.sync.dma_start(vsb[:], vrow[:])
            nc.sync.dma_start(isb[:], irow[:])
            nc.gpsimd.partition_broadcast(vb[:], vsb[:])
            nc.gpsimd.partition_broadcast(ib[:], isb[:])

            new2 = [pool.tile([128, 512], F32, tag=f"n2{t}", name=f"n2{t}") for t in range(2)]
            for mt in range(2):
                ps = psum.tile([128, 512], F32, tag="ps", name="ps")
                for kt in range(2):
                    nc.tensor.matmul(ps[:], st[kt][:, mt * 128:(mt + 1) * 128],
                                     xs[kt][:], start=(kt == 0), stop=(kt == 1))
                nc.vector.tensor_scalar(new2[mt][:], ps[:], rc[:, mt:mt + 1],
                                        None, ALU.mult)
            # allgather new2 -> window rows for L3 (device-chained)
            bG_in = dram.tile([256, 512], F32, name="bG_in")
            bG_out = dram.tile([8, 256, 512], F32, name="bG_out")
            for t in range(2):
                nc.sync.dma_start(bG_in[t * 128:(t + 1) * 128, :], new2[t][:])
            nc.gpsimd.collective_compute(
                "AllGather", ALU.bypass, replica_groups=RG,
                ins=[bG_in.opt()], outs=[bG_out.opt()])
            for rt in range(2):
                n2full = work.tile([128, 64, 64], F32, tag="n2full",
                                   name="n2full", bufs=1)
                n2fv = n2full[:].rearrange("p a b -> p (a b)")
                for k in range(NCORES):
                    nc.sync.dma_start(
                        n2fv[:, k * 512:(k + 1) * 512],
                        bG_out[k, rt * 128:(rt + 1) * 128, :])
                n2wt = work.tile([128, L3_W, 64], F32, tag="n2wt",
                                 name="n2wt", bufs=1)
                nc.gpsimd.ap_gather(n2wt[:], n2full[:], iw[:], channels=128,
                                    num_elems=64, d=64, num_idxs=L3_W)
                nc.sync.dma_start(
                    o_n2w[rt * 128:(rt + 1) * 128, :],
                    n2wt[:].rearrange("p a b -> p (a b)"))
            # transpose new2 -> n2T [4 x [128,256]]
            n2T = [pool.tile([128, 256], F32, tag=f"n2T{t}", name=f"n2T{t}") for t in range(4)]
            for ct in range(4):
                for rt in range(2):
                    pst = psum.tile([128, 128], F32, tag="tp", name="tp")
                    nc.tensor.transpose(
                        pst[:], new2[rt][:, ct * 128:(ct + 1) * 128], ident32[:])
                    nc.any.tensor_copy(n2T[ct][:, rt * 128:(rt + 1) * 128], pst[:])
            # masked = n2T*valid + inv ; ln
            ln2T = [pool.tile([128, 256], F32, tag=f"ln2T{t}", name=f"ln2T{t}") for t in range(4)]
            sx3P = psumS.tile([1, 256], F32, tag="sx3", name="sx3")
            for ct in range(4):
                msk = work.tile([128, 256], F32, tag="msk", name="msk")
                nc.vector.tensor_tensor(msk[:], n2T[ct][:], vb[:], ALU.mult)
                nc.vector.tensor_tensor(msk[:], msk[:], ib[:], ALU.add)
                nc.scalar.activation(ln2T[ct][:], msk[:], AF.Ln)
                prod = work.tile([128, 256], F32, tag="prod", name="prod")
                nc.vector.tensor_tensor(prod[:], n2T[ct][:], ln2T[ct][:], ALU.mult)
                nc.tensor.matmul(sx3P[:], ones32[:], prod[:],
                                 start=(ct == 0), stop=(ct == 3))
            sx3sb = work.tile([1, 256], F32, tag="sx3sb", name="sx3sb")
            nc.any.tensor_copy(sx3sb[:], sx3P[:])
            bS3_in = dram.tile([1, 256], F32, name="bS3_in")
            bS3_out = dram.tile([1, 256], F32, name="bS3_out")
            nc.sync.dma_start(bS3_in[:], sx3sb[:])
            nc.gpsimd.collective_compute(
                "AllReduce", ALU.add, replica_groups=RG,
                ins=[bS3_in.opt()], outs=[bS3_out.opt()])
            nc.sync.dma_start(o_kl3[256:257, :], bS3_out[:])
            bC3_in = dram.tile([256, 256], F32, name="bC3_in")
            bC3_out = dram.tile([256, 256], F32, name="bC3_out")
            for it in range(2):
                psC = psum.tile([128, 256], F32, tag="psC", name="psC")
                for kt in range(4):
                    nc.tensor.matmul(psC[:], n2T[kt][:, it * 128:(it + 1) * 128],
                                     ln2T[kt][:], start=(kt == 0), stop=(kt == 3))
                c3t = work.tile([128, 256], F32, tag="c3t", name="c3t")
                nc.any.tensor_copy(c3t[:], psC[:])
                nc.sync.dma_start(bC3_in[it * 128:(it + 1) * 128, :], c3t[:])
            nc.gpsimd.collective_compute(
                "AllReduce", ALU.add, replica_groups=RG,
                ins=[bC3_in.opt()], outs=[bC3_out.opt()])
            nc.sync.dma_start(o_kl3[0:256, :], bC3_out[:])

    nc.finalize()
    return nc, ["x2s", "sel2T", "icnt2", "vrow", "irow", "iwin"], \
        ["n2w", "klpack3"]


def _build_l3():
    nc, tile, mybir = _build_common()
    from concourse.masks import make_identity
    F32, F16 = mybir.dt.float32, mybir.dt.float16
    I16, U32 = mybir.dt.int16, mybir.dt.uint32
    ALU, AX, AF = mybir.AluOpType, mybir.AxisListType, mybir.ActivationFunctionType

    n2w = nc.dram_tensor("n2w", [256, L3_W * 64], F32, kind="ExternalInput")
    selT = nc.dram_tensor("sel3T", [256, 256], mybir.dt.uint8,
                          kind="ExternalInput")
    icnt = nc.dram_tensor("icnt3", [256, 1], F32, kind="ExternalInput")
    bias = nc.dram_tensor("biasv", [256, 1], F32, kind="ExternalInput")
    idxyc = nc.dram_tensor("idxyc", [128, 2], I16, kind="ExternalInput")
    idxyd = nc.dram_tensor("idxyd", [128, 2], I16, kind="ExternalInput")
    wyr = nc.dram_tensor("wy", [1, 32], F32, kind="ExternalInput")
    idxxc = nc.dram_tensor("idxxc", [128, 512], I16, kind="ExternalInput")
    idxxd = nc.dram_tensor("idxxd", [128, 512], I16, kind="ExternalInput")
    wxr = nc.dram_tensor("wx", [1, 256], F32, kind="ExternalInput")
    o_lab = nc.dram_tensor("lab", [8, 128, 64], F32, kind="ExternalOutput")
    RG = [list(range(NCORES))]

    W = L3_W * 64
    with tile.TileContext(nc) as tc:
        with tc.tile_pool(name="sb", bufs=1) as pool, \
             tc.tile_pool(name="work", bufs=2) as work, \
             tc.tile_pool(name="big", bufs=1) as big, \
             tc.tile_pool(name="psum", bufs=2, space="PSUM") as psum, \
             tc.tile_pool(name="dram", bufs=1, space="DRAM") as dram:
            ident32 = pool.tile([128, 128], F32, tag="id32", name="id32")
            make_identity(nc, ident32[:])
            nw = [pool.tile([128, W], F32, tag=f"nw{t}", name=f"nw{t}") for t in range(2)]
            st = [pool.tile([128, 256], F32, tag=f"st{t}", name=f"st{t}") for t in range(2)]
            stu = [pool.tile([128, 256], mybir.dt.uint8, tag=f"stu{t}",
                             name=f"stu{t}") for t in range(2)]
            for t in range(2):
                nc.sync.dma_start(nw[t][:], n2w[t * 128:(t + 1) * 128, :])
                nc.sync.dma_start(stu[t][:], selT[t * 128:(t + 1) * 128, :])
                nc.any.tensor_copy(st[t][:], stu[t][:])
            cnt = pool.tile([128, 2], F32, tag="cnt", name="cnt")
            nc.sync.dma_start(cnt[:], icnt[:].rearrange("(a p) b -> p (a b)", a=2))
            rc = pool.tile([128, 2], F32, tag="rc", name="rc")
            nc.vector.reciprocal(rc[:], cnt[:])
            bv = pool.tile([128, 2], F32, tag="bv", name="bv")
            nc.sync.dma_start(bv[:], bias[:].rearrange("(a p) b -> p (a b)", a=2))
            iyc = pool.tile([128, 2], I16, tag="iyc", name="iyc")
            iyd = pool.tile([128, 2], I16, tag="iyd", name="iyd")
            ixc = pool.tile([128, 512], I16, tag="ixc", name="ixc")
            ixd = pool.tile([128, 512], I16, tag="ixd", name="ixd")
            for t_, s_ in ((iyc, idxyc), (iyd, idxyd), (ixc, idxxc), (ixd, idxxd)):
                nc.sync.dma_start(t_[:], s_[:])
            wyt = pool.tile([128, 32], F32, tag="wyt", name="wyt")
            wxt = pool.tile([128, 256], F32, tag="wxt", name="wxt")
            wsb = work.tile([1, 32], F32, tag="wsb", name="wsb")
            nc.sync.dma_start(wsb[:], wyr[:])
            nc.gpsimd.partition_broadcast(wyt[:], wsb[:])
            wsb2 = work.tile([1, 256], F32, tag="wsb2", name="wsb2")
            nc.sync.dma_start(wsb2[:], wxr[:])
            nc.gpsimd.partition_broadcast(wxt[:], wsb2[:])

            up = [big.tile([128, 8192, 1], F32, tag=f"up{t}", name=f"up{t}")
                  for t in range(2)]
            for mt in range(2):
                n3 = work.tile([128, W], F32, tag="n3", name="n3")
                for half, (c0, c1) in enumerate(((0, 512), (512, W))):
                    ps = psum.tile([128, c1 - c0], F32, tag=f"ps{half}", name=f"ps{half}")
                    for kt in range(2):
                        nc.tensor.matmul(ps[:],
                                         st[kt][:, mt * 128:(mt + 1) * 128],
                                         nw[kt][:, c0:c1],
                                         start=(kt == 0), stop=(kt == 1))
                    nc.vector.tensor_scalar(n3[:, c0:c1], ps[:],
                                            rc[:, mt:mt + 1], None, ALU.mult)
                nc.vector.tensor_scalar(n3[:], n3[:], bv[:, mt:mt + 1], None,
                                        ALU.add)
                # y-interp via gather: [128,10,64] -> c,d [128,32,64]
                yc = work.tile([128, 32, 64], F32, tag="yc", name="yc")
                yd = work.tile([128, 32, 64], F32, tag="yd", name="yd")
                ydr = work.tile([128, 2048, 1], F32, tag="ydr", name="ydr")
                n3v = n3[:].rearrange("p (y x) -> p y x", y=L3_W)
                nc.gpsimd.ap_gather(yc[:], n3v, iyc[:], channels=128,
                                    num_elems=L3_W, d=64, num_idxs=32)
                nc.gpsimd.ap_gather(yd[:], n3v, iyd[:], channels=128,
                                    num_elems=L3_W, d=64, num_idxs=32)
                yc3 = yc[:]
                yd3 = yd[:]
                ydr3 = ydr[:].rearrange("p (y x) o -> p y (x o)", y=32)
                wy3 = wyt[:, :, None].broadcast_to([128, 32, 64])
                nc.vector.tensor_tensor(ydr3, yd3, yc3, ALU.subtract)
                nc.vector.tensor_tensor(ydr3, ydr3, wy3, ALU.mult)
                nc.vector.tensor_tensor(ydr3, ydr3, yc3, ALU.add)
                # x-interp via gather on [128, 2048, 1] -> [128, 8192]
                xc = big.tile([128, 8192, 1], F32, tag="xc", name="xc")
                xd = up[mt]
                nc.gpsimd.ap_gather(xc[:], ydr[:], ixc[:], channels=128,
                                    num_elems=2048, d=1, num_idxs=8192)
                nc.gpsimd.ap_gather(xd[:], ydr[:], ixd[:], channels=128,
                                    num_elems=2048, d=1, num_idxs=8192)
                xc3 = xc[:].rearrange("p (y j) o -> p y (j o)", y=32)
                xd3 = xd[:].rearrange("p (y j) o -> p y (j o)", y=32)
                wx3 = wxt[:, None, :].broadcast_to([128, 32, 256])
                nc.vector.tensor_tensor(xd3, xd3, xc3, ALU.subtract)
                nc.vector.tensor_tensor(xd3, xd3, wx3, ALU.mult)
                nc.vector.tensor_tensor(xd3, xd3, xc3, ALU.add)
            # transpose + argmax
            lab = pool.tile([128, 64], F32, tag="lab", name="lab")
            upf = [u[:].rearrange("p n o -> p (n o)") for u in up]
            for pt in range(64):
                sc = work.tile([128, 256], F32, tag="sc", name="sc")
                for mt in range(2):
                    pst = psum.tile([128, 128], F32, tag="tp", name="tp")
                    nc.tensor.transpose(
                        pst[:], upf[mt][:, pt * 128:(pt + 1) * 128], ident32[:])
                    nc.any.tensor_copy(sc[:, mt * 128:(mt + 1) * 128], pst[:])
                mx = work.tile([128, 8], F32, tag="mx", name="mx")
                nc.vector.max(mx[:], sc[:])
                mi = work.tile([128, 8], U32, tag="mi", name="mi")
                nc.vector.max_index(mi[:], mx[:], sc[:])
                nc.vector.tensor_copy(lab[:, pt:pt + 1], mi[:, 0:1])
            bL_in = dram.tile([128, 64], F32, name="bL_in")
            bL_out = dram.tile([8, 128, 64], F32, name="bL_out")
            nc.sync.dma_start(bL_in[:], lab[:])
            nc.gpsimd.collective_compute(
                "AllGather", ALU.bypass, replica_groups=RG,
                ins=[bL_in.opt()], outs=[bL_out.opt()])
            nc.sync.dma_start(o_lab[:], bL_out[:])

    nc.finalize()
    return nc, ["n2w", "sel3T", "icnt3", "biasv", "idxyc", "idxyd", "wy",
                "idxxc", "idxxd", "wx"], ["lab"]


def _emit_l23(nc, tile, mybir, tc, x2s, klp, iwin, idxyc, idxyd, wyr,
              idxxc, idxxd, wxr, o_pack):
    """Fused L2+greedy2+greedy3+L3: everything after L1 in ONE program.

    Greedy selection runs on-device in SCATTERED form: instead of compacting
    representative rows to the top (reference), rows stay at their anchor
    index and an active-mask a[] marks representatives. Compaction is order-
    preserving, so all downstream math is value-identical; the final argmax
    indices are remapped on the host via a cumsum LUT over a3.
    """
    from concourse.masks import make_identity
    F32, F16 = mybir.dt.float32, mybir.dt.float16
    I16, U8, U32 = mybir.dt.int16, mybir.dt.uint8, mybir.dt.uint32
    ALU, AX, AF = mybir.AluOpType, mybir.AxisListType, mybir.ActivationFunctionType
    RG = [list(range(NCORES))]
    W = L3_W * 64

    def klprep(tc, pool, work, psT, ident32, Crows, sxrow, tag):
        """Build U = (sx_i + sx_j - C - C^T < 1.8) as two [128,256] 0/1 tiles.
        (0.5*t < 0.9 <=> t < 1.8 exactly: *0.5 is a power-of-2 scale.)"""
        Ct = [pool.tile([128, 256], F32, tag=f"Ct{tag}{t}", name=f"Ct{tag}{t}")
              for t in range(2)]
        for bi in range(2):
            for bj in range(2):
                pst = psT.tile([128, 128], F32, tag="tpK", name=f"tpK{tag}")
                nc.tensor.transpose(
                    pst[:], Crows[bi][:, bj * 128:(bj + 1) * 128], ident32[:])
                nc.any.tensor_copy(Ct[bj][:, bi * 128:(bi + 1) * 128], pst[:])
        # sx row -> broadcast + column
        sxb = pool.tile([128, 256], F32, tag=f"sxb{tag}", name=f"sxb{tag}")
        nc.gpsimd.partition_broadcast(sxb[:], sxrow[:])
        zp = work.tile([128, 256], F32, tag="zpK", name=f"zpK{tag}")
        nc.gpsimd.memset(zp[:], 0.0)
        nc.vector.tensor_copy(zp[0:1, :], sxrow[:])
        sxcol = [pool.tile([128, 1], F32, tag=f"sxc{tag}{t}",
                           name=f"sxc{tag}{t}") for t in range(2)]
        for bj in range(2):
            pst = psT.tile([128, 128], F32, tag="tpK", name=f"tpK2{tag}")
            nc.tensor.transpose(
                pst[:], zp[:, bj * 128:(bj + 1) * 128], ident32[:])
            nc.any.tensor_copy(sxcol[bj][:], pst[:, 0:1])
        U = [pool.tile([128, 256], F32, tag=f"U{tag}{t}", name=f"U{tag}{t}")
             for t in range(2)]
        for rt in range(2):
            t_ = work.tile([128, 256], F32, tag="tK", name=f"tK{tag}")
            nc.vector.tensor_scalar(t_[:], sxb[:], sxcol[rt][:], None, ALU.add)
            nc.vector.tensor_tensor(t_[:], t_[:], Crows[rt][:], ALU.subtract)
            nc.vector.tensor_tensor(t_[:], t_[:], Ct[rt][:], ALU.subtract)
            nc.vector.tensor_scalar(U[rt][:], t_[:], float(THR2), None,
                                    ALU.is_lt)
        return U

    def greedy(tc, pool, work, ident32, U, validrow, apad, tag):
        """Sequential greedy: apad[0:1,:] <- active mask. U: 2x[128,256] 0/1
        (already column-masked by valid). validrow: [1,256] 0/1 tile AP."""
        matched = pool.tile([1, 256], F32, tag=f"mt{tag}", name=f"mt{tag}")
        nc.gpsimd.memset(matched[:], 0.0)
        nc.gpsimd.memset(apad[:], 0.0)
        with tc.tile_pool(name=f"psE{tag}", bufs=4, space="PSUM") as psE:
            for i in range(256):
                ps = psE.tile([1, 256], F32, tag="ext", name=f"ext{tag}")
                nc.tensor.matmul(ps[:], ident32[:, i % 128:i % 128 + 1],
                                 U[i // 128][:], start=True, stop=True)
                # a_i = valid[i] - matched[i]  (matched <= valid always)
                nc.vector.tensor_tensor(
                    apad[0:1, i:i + 1], validrow[0:1, i:i + 1],
                    matched[0:1, i:i + 1], ALU.subtract)
                srow = work.tile([1, 256], F32, tag=f"sr{tag}",
                                 name=f"sr{tag}")
                nc.vector.tensor_scalar(srow[:], ps[0:1, :],
                                        apad[0:1, i:i + 1], None, ALU.mult)
                nc.vector.tensor_tensor(matched[0:1, :], matched[0:1, :],
                                        srow[:], ALU.max)

    if True:
        with tc.tile_pool(name="sb", bufs=1) as pool, \
             tc.tile_pool(name="work", bufs=2) as work, \
             tc.tile_pool(name="big", bufs=1) as big, \
             tc.tile_pool(name="dram", bufs=1, space="DRAM") as dram:
            ident32 = pool.tile([128, 128], F32, tag="id32", name="id32")
            make_identity(nc, ident32[:])
            ones32 = pool.tile([128, 1], F32, tag="o32", name="o32")
            nc.gpsimd.memset(ones32[:], 1.0)
            onesrow = pool.tile([1, 256], F32, tag="or", name="or")
            nc.gpsimd.memset(onesrow[:], 1.0)
            iw = pool.tile([128, 1], I16, tag="iw", name="iw")
            nc.sync.dma_start(iw[:], iwin[:])
            xs = [pool.tile([128, 512], F32, tag=f"xs{t}", name=f"xs{t}")
                  for t in range(2)]
            for t in range(2):
                nc.sync.dma_start(xs[t][:], x2s[t * 128:(t + 1) * 128, :])
            C1 = [work.tile([128, 256], F32, tag=f"C1_{t}", name=f"C1_{t}",
                            bufs=1) for t in range(2)]
            for t in range(2):
                nc.sync.dma_start(C1[t][:], klp[t * 128:(t + 1) * 128, :])
            sxr = work.tile([1, 256], F32, tag="sxr", name="sxr", bufs=1)
            nc.sync.dma_start(sxr[:], klp[256:257, :])

            # ---- greedy 2 (valid == ones) ----
            a2pad = pool.tile([128, 256], F32, tag="a2p", name="a2p")
            with tc.tile_pool(name="psT", bufs=2, space="PSUM") as psT:
                U2 = klprep(tc, pool, work, psT, ident32, C1, sxr, "2")
            greedy(tc, pool, work, ident32, U2, onesrow, a2pad, "2")

            a2bc = pool.tile([128, 256], F32, tag="a2bc", name="a2bc")
            nc.gpsimd.partition_broadcast(a2bc[:], a2pad[0:1, :])
            ib2 = pool.tile([128, 256], F32, tag="ib2", name="ib2")
            nc.vector.tensor_scalar(ib2[:], a2bc[:], -1.0, 1.0, ALU.mult,
                                    ALU.add)
            # selT2 = U2 * a2[free]; new2 = selT2^T @ X2 / cnt
            selT2 = [pool.tile([128, 256], F32, tag=f"sT2_{t}",
                               name=f"sT2_{t}") for t in range(2)]
            for t in range(2):
                nc.vector.tensor_tensor(selT2[t][:], U2[t][:], a2bc[:],
                                        ALU.mult)
            new2 = [pool.tile([128, 512], F32, tag=f"n2{t}", name=f"n2{t}")
                    for t in range(2)]
            rc2 = [pool.tile([128, 1], F32, tag=f"rc2{t}", name=f"rc2{t}")
                   for t in range(2)]
            with tc.tile_pool(name="psB", bufs=2, space="PSUM") as psum, \
                 tc.tile_pool(name="psS", bufs=1, space="PSUM") as psumS:
                for mt in range(2):
                    cc = psumS.tile([128, 1], F32, tag="cc", name="cc")
                    for kt in range(2):
                        nc.tensor.matmul(
                            cc[:], selT2[kt][:, mt * 128:(mt + 1) * 128],
                            ones32[:], start=(kt == 0), stop=(kt == 1))
                    nc.vector.tensor_scalar(rc2[mt][:], cc[:], 1.0, None,
                                            ALU.max)
                    nc.vector.reciprocal(rc2[mt][:], rc2[mt][:])
                for mt in range(2):
                    ps = psum.tile([128, 512], F32, tag="psN", name="psN")
                    for kt in range(2):
                        nc.tensor.matmul(
                            ps[:], selT2[kt][:, mt * 128:(mt + 1) * 128],
                            xs[kt][:], start=(kt == 0), stop=(kt == 1))
                    nc.vector.tensor_scalar(new2[mt][:], ps[:], rc2[mt][:],
                                            None, ALU.mult)

                # allgather new2 -> window rows for L3 part
                bG_in = dram.tile([256, 512], F32, name="bG_in")
                bG_out = dram.tile([8, 256, 512], F32, name="bG_out")
                for t in range(2):
                    nc.sync.dma_start(bG_in[t * 128:(t + 1) * 128, :],
                                      new2[t][:])
                nc.gpsimd.collective_compute(
                    "AllGather", ALU.bypass, replica_groups=RG,
                    ins=[bG_in.opt()], outs=[bG_out.opt()])
                nw = [pool.tile([128, W], F32, tag=f"nw{t}", name=f"nw{t}")
                      for t in range(2)]
                for rt in range(2):
                    n2full = big.tile([128, 64, 64], F32, tag="bigsc",
                                      name="n2full")
                    n2fv = n2full[:].rearrange("p a b -> p (a b)")
                    for k in range(NCORES):
                        nc.sync.dma_start(
                            n2fv[:, k * 512:(k + 1) * 512],
                            bG_out[k, rt * 128:(rt + 1) * 128, :])
                    n2wt = nw[rt][:].rearrange("p (a b) -> p a b", a=L3_W)
                    nc.gpsimd.ap_gather(n2wt, n2full[:], iw[:], channels=128,
                                        num_elems=64, d=64, num_idxs=L3_W)

                # n2T, masked ln, sx3/C3 partials -> single [257,256] AllReduce
                n2T = [pool.tile([128, 256], F32, tag=f"n2T{t}",
                                 name=f"n2T{t}") for t in range(4)]
                for ct in range(4):
                    for rt in range(2):
                        pst = psum.tile([128, 128], F32, tag="tp", name="tp")
                        nc.tensor.transpose(
                            pst[:], new2[rt][:, ct * 128:(ct + 1) * 128],
                            ident32[:])
                        nc.any.tensor_copy(
                            n2T[ct][:, rt * 128:(rt + 1) * 128], pst[:])
                ln2T = [work.tile([128, 256], F32, tag=f"ln2T{t}",
                                  name=f"ln2T{t}", bufs=1) for t in range(4)]
                sx3P = psumS.tile([1, 256], F32, tag="sx3", name="sx3")
                for ct in range(4):
                    msk = work.tile([128, 256], F32, tag="msk", name="msk")
                    nc.vector.tensor_tensor(msk[:], n2T[ct][:], a2bc[:],
                                            ALU.mult)
                    nc.vector.tensor_tensor(msk[:], msk[:], ib2[:], ALU.add)
                    nc.scalar.activation(ln2T[ct][:], msk[:], AF.Ln)
                    prod = work.tile([128, 256], F32, tag="prod", name="prod")
                    nc.vector.tensor_tensor(prod[:], n2T[ct][:], ln2T[ct][:],
                                            ALU.mult)
                    nc.tensor.matmul(sx3P[:], ones32[:], prod[:],
                                     start=(ct == 0), stop=(ct == 3))
                bK_in = dram.tile([257, 256], F32, name="bK_in")
                bK_out = dram.tile([257, 256], F32, name="bK_out")
                sx3sb = work.tile([1, 256], F32, tag="sx3sb", name="sx3sb")
                nc.any.tensor_copy(sx3sb[:], sx3P[:])
                nc.sync.dma_start(bK_in[256:257, :], sx3sb[:])
                for it in range(2):
                    psC = psum.tile([128, 256], F32, tag="psC", name="psC")
                    for kt in range(4):
                        nc.tensor.matmul(
                            psC[:], n2T[kt][:, it * 128:(it + 1) * 128],
                            ln2T[kt][:], start=(kt == 0), stop=(kt == 3))
                    c3t = work.tile([128, 256], F32, tag="c3t", name="c3t")
                    nc.any.tensor_copy(c3t[:], psC[:])
                    nc.sync.dma_start(bK_in[it * 128:(it + 1) * 128, :],
                                      c3t[:])
                nc.gpsimd.collective_compute(
                    "AllReduce", ALU.add, replica_groups=RG,
                    ins=[bK_in.opt()], outs=[bK_out.opt()])

            # ---- greedy 3 (valid == a2) ----
            C3 = [work.tile([128, 256], F32, tag=f"C3_{t}", name=f"C3_{t}",
                            bufs=1) for t in range(2)]
            for t in range(2):
                nc.sync.dma_start(C3[t][:], bK_out[t * 128:(t + 1) * 128, :])
            sx3r = work.tile([1, 256], F32, tag="sx3r", name="sx3r", bufs=1)
            nc.sync.dma_start(sx3r[:], bK_out[256:257, :])
            a3pad = pool.tile([128, 256], F32, tag="a3p", name="a3p")
            with tc.tile_pool(name="psT3", bufs=2, space="PSUM") as psT3:
                U3 = klprep(tc, pool, work, psT3, ident32, C3, sx3r, "3")
                for t in range(2):
                    nc.vector.tensor_tensor(U3[t][:], U3[t][:], a2bc[:],
                                            ALU.mult)
            greedy(tc, pool, work, ident32, U3, a2pad[0:1, :], a3pad, "3")

            # a3 columns, sel3T = transpose(U3 * a3col), cnt3, bias
            a3col = [pool.tile([128, 1], F32, tag=f"a3c{t}", name=f"a3c{t}")
                     for t in range(2)]
            sel3T = [pool.tile([128, 256], F32, tag=f"sT3_{t}",
                               name=f"sT3_{t}") for t in range(2)]
            rc3 = [pool.tile([128, 1], F32, tag=f"rc3{t}", name=f"rc3{t}")
                   for t in range(2)]
            bv = [pool.tile([128, 1], F32, tag=f"bv{t}", name=f"bv{t}")
                  for t in range(2)]
            up = [big.tile([128, 8192, 1], F32, tag=f"up{t}", name=f"up{t}")
                  for t in range(2)]
            with tc.tile_pool(name="psD", bufs=2, space="PSUM") as psum, \
                 tc.tile_pool(name="psS3", bufs=1, space="PSUM") as psumS:
                for bj in range(2):
                    pst = psum.tile([128, 128], F32, tag="tpD", name="tp3")
                    nc.tensor.transpose(
                        pst[:], a3pad[:, bj * 128:(bj + 1) * 128], ident32[:])
                    nc.any.tensor_copy(a3col[bj][:], pst[:, 0:1])
                S3 = [work.tile([128, 256], F32, tag=f"S3_{t}",
                                name=f"S3_{t}", bufs=1) for t in range(2)]
                for t in range(2):
                    nc.vector.tensor_scalar(S3[t][:], U3[t][:], a3col[t][:],
                                            None, ALU.mult)
                for bi in range(2):
                    for bj in range(2):
                        pst = psum.tile([128, 128], F32, tag="tpD",
                                        name="tp3b")
                        nc.tensor.transpose(
                            pst[:], S3[bi][:, bj * 128:(bj + 1) * 128],
                            ident32[:])
                        nc.any.tensor_copy(
                            sel3T[bj][:, bi * 128:(bi + 1) * 128], pst[:])
                for mt in range(2):
                    cc = psumS.tile([128, 1], F32, tag="cc3", name="cc3")
                    for kt in range(2):
                        nc.tensor.matmul(
                            cc[:], sel3T[kt][:, mt * 128:(mt + 1) * 128],
                            ones32[:], start=(kt == 0), stop=(kt == 1))
                    nc.vector.tensor_scalar(rc3[mt][:], cc[:], 1.0, None,
                                            ALU.max)
                    nc.vector.reciprocal(rc3[mt][:], rc3[mt][:])
                    nc.vector.tensor_scalar(bv[mt][:], a3col[mt][:], -1.0,
                                            float(-NEG_BIG), ALU.add,
                                            ALU.mult)

                # ---- L3: new3 window, upsample, argmax ----
                wyt = pool.tile([128, 32], F32, tag="wyt", name="wyt")
                wxt = pool.tile([128, 256], F32, tag="wxt", name="wxt")
                wsb = work.tile([1, 32], F32, tag="wsb", name="wsb")
                nc.sync.dma_start(wsb[:], wyr[:])
                nc.gpsimd.partition_broadcast(wyt[:], wsb[:])
                wsb2 = work.tile([1, 256], F32, tag="wsb2", name="wsb2")
                nc.sync.dma_start(wsb2[:], wxr[:])
                nc.gpsimd.partition_broadcast(wxt[:], wsb2[:])
                iyc = pool.tile([128, 2], I16, tag="iyc", name="iyc")
                iyd = pool.tile([128, 2], I16, tag="iyd", name="iyd")
                ixc = pool.tile([128, 512], I16, tag="ixc", name="ixc")
                ixd = pool.tile([128, 512], I16, tag="ixd", name="ixd")
                for t_, s_ in ((iyc, idxyc), (iyd, idxyd), (ixc, idxxc),
                               (ixd, idxxd)):
                    nc.sync.dma_start(t_[:], s_[:])

                for mt in range(2):
                    n3 = work.tile([128, W], F32, tag="n3", name="n3",
                                   bufs=1)
                    for half, (c0, c1) in enumerate(((0, 512), (512, W))):
                        ps = psum.tile([128, 512], F32, tag="psH",
                                       name=f"ps{half}")
                        psv = ps[:, 0:c1 - c0]
                        for kt in range(2):
                            nc.tensor.matmul(
                                psv, sel3T[kt][:, mt * 128:(mt + 1) * 128],
                                nw[kt][:, c0:c1],
                                start=(kt == 0), stop=(kt == 1))
                        nc.vector.tensor_scalar(n3[:, c0:c1], psv,
                                                rc3[mt][:], None, ALU.mult)
                    nc.vector.tensor_scalar(n3[:], n3[:], bv[mt][:], None,
                                            ALU.add)
                    yc = work.tile([128, 32, 64], F32, tag="yc", name="yc",
                                   bufs=1)
                    yd = work.tile([128, 32, 64], F32, tag="yd", name="yd",
                                   bufs=1)
                    ydr = work.tile([128, 2048, 1], F32, tag="ydr",
                                    name="ydr", bufs=1)
                    n3v = n3[:].rearrange("p (y x) -> p y x", y=L3_W)
                    nc.gpsimd.ap_gather(yc[:], n3v, iyc[:], channels=128,
                                        num_elems=L3_W, d=64, num_idxs=32)
                    nc.gpsimd.ap_gather(yd[:], n3v, iyd[:], channels=128,
                                        num_elems=L3_W, d=64, num_idxs=32)
                    ydr3 = ydr[:].rearrange("p (y x) o -> p y (x o)", y=32)
                    wy3 = wyt[:, :, None].broadcast_to([128, 32, 64])
                    nc.vector.tensor_tensor(ydr3, yd[:], yc[:], ALU.subtract)
                    nc.vector.tensor_tensor(ydr3, ydr3, wy3, ALU.mult)
                    nc.vector.tensor_tensor(ydr3, ydr3, yc[:], ALU.add)
                    xc = big.tile([128, 8192, 1], F32, tag="bigsc",
                                  name="xc")
                    xd = up[mt]
                    nc.gpsimd.ap_gather(xc[:], ydr[:], ixc[:], channels=128,
                                        num_elems=2048, d=1, num_idxs=8192)
                    nc.gpsimd.ap_gather(xd[:], ydr[:], ixd[:], channels=128,
                                        num_elems=2048, d=1, num_idxs=8192)
                    xc3 = xc[:].rearrange("p (y j) o -> p y (j o)", y=32)
                    xd3 = xd[:].rearrange("p (y j) o -> p y (j o)", y=32)
                    wx3 = wxt[:, None, :].broadcast_to([128, 32, 256])
                    nc.vector.tensor_tensor(xd3, xd3, xc3, ALU.subtract)
                    nc.vector.tensor_tensor(xd3, xd3, wx3, ALU.mult)
                    nc.vector.tensor_tensor(xd3, xd3, xc3, ALU.add)
                # transpose + argmax (labels as u8 scattered anchor idx)
                pack = pool.tile([128, 66], U8, tag="pack", name="pack")
                upf = [u[:].rearrange("p n o -> p (n o)") for u in up]
                for pt in range(64):
                    sc = work.tile([128, 256], F32, tag="sc", name="sc")
                    for mt in range(2):
                        pst = psum.tile([128, 128], F32, tag="tpD", name="tpA")
                        nc.tensor.transpose(
                            pst[:], upf[mt][:, pt * 128:(pt + 1) * 128],
                            ident32[:])
                        nc.any.tensor_copy(sc[:, mt * 128:(mt + 1) * 128],
                                           pst[:])
                    mx = work.tile([128, 8], F32, tag="mx", name="mx")
                    nc.vector.max(mx[:], sc[:])
                    mi = work.tile([128, 8], U32, tag="mi", name="mi")
                    nc.vector.max_index(mi[:], mx[:], sc[:])
                    nc.vector.tensor_copy(pack[:, pt:pt + 1], mi[:, 0:1])
                nc.vector.tensor_copy(pack[:, 64:65], a3col[0][:])
                nc.vector.tensor_copy(pack[:, 65:66], a3col[1][:])
                bP_in = dram.tile([128, 66], U8, name="bP_in")
                bP_out = dram.tile([8, 128, 66], U8, name="bP_out")
                nc.sync.dma_start(bP_in[:], pack[:])
                nc.gpsimd.collective_compute(
                    "AllGather", ALU.bypass, replica_groups=RG,
                    ins=[bP_in.opt()], outs=[bP_out.opt()])
                nc.sync.dma_start(o_pack[:], bP_out[:])


def _build_l23():
    nc, tile, mybir = _build_common()
    F32, I16, U8 = mybir.dt.float32, mybir.dt.int16, mybir.dt.uint8
    x2s = nc.dram_tensor("x2s", [256, 512], F32, kind="ExternalInput")
    klp = nc.dram_tensor("klp", [257, 256], F32, kind="ExternalInput")
    iwin = nc.dram_tensor("iwin", [128, 1], I16, kind="ExternalInput")
    idxyc = nc.dram_tensor("idxyc", [128, 2], I16, kind="ExternalInput")
    idxyd = nc.dram_tensor("idxyd", [128, 2], I16, kind="ExternalInput")
    wyr = nc.dram_tensor("wy", [1, 32], F32, kind="ExternalInput")
    idxxc = nc.dram_tensor("idxxc", [128, 512], I16, kind="ExternalInput")
    idxxd = nc.dram_tensor("idxxd", [128, 512], I16, kind="ExternalInput")
    wxr = nc.dram_tensor("wx", [1, 256], F32, kind="ExternalInput")
    o_pack = nc.dram_tensor("opack", [8, 128, 66], U8, kind="ExternalOutput")
    with tile.TileContext(nc) as tc:
        _emit_l23(nc, tile, mybir, tc, x2s, klp, iwin, idxyc, idxyd, wyr,
                  idxxc, idxxd, wxr, o_pack)
    nc.finalize()
    return nc, ["x2s", "klp", "iwin", "idxyc", "idxyd", "wy",
                "idxxc", "idxxd", "wx"], ["opack"]


def _build_l123():
    """Everything in ONE program: aggregation+merge0 (L1) then fused
    greedy/merge/upsample/argmax (L23), chained through internal DRAM."""
    nc, tile, mybir = _build_common()
    F32, I16, U8 = mybir.dt.float32, mybir.dt.int16, mybir.dt.uint8
    w64 = nc.dram_tensor("w64s", [8, 512, 4096], F32, kind="ExternalInput")
    w32 = nc.dram_tensor("w32s", [8, 256, 1024], F32, kind="ExternalInput")
    w16 = nc.dram_tensor("w16s", [8, 128, 256], F32, kind="ExternalInput")
    w8 = nc.dram_tensor("w8s", [8, 64, 64], F32, kind="ExternalInput")
    iwin = nc.dram_tensor("iwin", [128, 1], I16, kind="ExternalInput")
    idxyc = nc.dram_tensor("idxyc", [128, 2], I16, kind="ExternalInput")
    idxyd = nc.dram_tensor("idxyd", [128, 2], I16, kind="ExternalInput")
    wyr = nc.dram_tensor("wy", [1, 32], F32, kind="ExternalInput")
    idxxc = nc.dram_tensor("idxxc", [128, 512], I16, kind="ExternalInput")
    idxxd = nc.dram_tensor("idxxd", [128, 512], I16, kind="ExternalInput")
    wxr = nc.dram_tensor("wx", [1, 256], F32, kind="ExternalInput")
    o_pack = nc.dram_tensor("opack", [8, 128, 66], U8, kind="ExternalOutput")
    with tile.TileContext(nc) as tc:
        with tc.tile_pool(name="xfer", bufs=1, space="DRAM") as xfer:
            x2d = xfer.tile([256, 512], F32, name="x2d")
            klpd = xfer.tile([257, 256], F32, name="klpd")
            _emit_l1(nc, tile, mybir, tc, w64, w32, w16, w8, x2d, klpd)
            _emit_l23(nc, tile, mybir, tc, x2d, klpd, iwin, idxyc, idxyd,
                      wyr, idxxc, idxxd, wxr, o_pack)
    nc.finalize()
    return nc, ["w64s", "w32s", "w16s", "w8s", "iwin", "idxyc", "idxyd",
                "wy", "idxxc", "idxxd", "wx"], ["opack"]


# ------------------------------------------------------------------- runner
class _Runner:
    """Cached shard_map-jitted executor for a finalized Bass program
    (modeled on bass2jax.run_bass_via_pjrt, but reusable across calls)."""

    def __init__(self, nc):
        import jax
        import jax.numpy as jnp
        from jax.sharding import Mesh, PartitionSpec, NamedSharding
        from jax.experimental.shard_map import shard_map
        from concourse import bass2jax as b2j
        from concourse import mybir
        b2j.install_neuronx_cc_hook()
        self.jax = jax
        self.np_outs = []
        in_names, out_names, out_avals, zero_outs = [], [], [], []
        partition_name = (nc.partition_id_tensor.name
                          if nc.partition_id_tensor else None)
        in_shapes = []
        for alloc in nc.m.functions[0].allocations:
            if not isinstance(alloc, mybir.MemoryLocationSet):
                continue
            name = alloc.memorylocations[0].name
            if alloc.kind == "ExternalInput":
                if name != partition_name:
                    in_names.append(name)
                    in_shapes.append((tuple(alloc.tensor_shape),
                                      mybir.dt.np(alloc.dtype)))
            elif alloc.kind == "ExternalOutput":
                shape = tuple(alloc.tensor_shape)
                dtype = mybir.dt.np(alloc.dtype)
                out_names.append(name)
                out_avals.append(jax.core.ShapedArray(shape, dtype))
                zero_outs.append(np.zeros(shape, dtype))
        self.in_names, self.out_names = in_names, out_names
        self.zero_outs = zero_outs
        n_params = len(in_names)
        bind_in_names = tuple(in_names + out_names +
                              ([partition_name] if partition_name else []))

        def _body(*args):
            operands = list(args)
            if partition_name is not None:
                operands.append(b2j.partition_id_tensor())
            outs = b2j._bass_exec_p.bind(
                *operands,
                out_avals=tuple(out_avals),
                in_names=bind_in_names,
                out_names=tuple(out_names),
                lowering_input_output_aliases=(),
                sim_require_finite=False,
                sim_require_nnan=False,
                nc=nc,
            )
            return tuple(outs)

        devices = jax.devices()[:NCORES]
        mesh = Mesh(np.asarray(devices), ("core",))
        n_outs = len(out_names)
        in_specs = (PartitionSpec("core"),) * (n_params + n_outs)
        out_specs = (PartitionSpec("core"),) * n_outs
        donate = tuple(range(n_params, n_params + n_outs))

        def _mk_jit():
            return jax.jit(
                shard_map(_body, mesh=mesh, in_specs=in_specs,
                          out_specs=out_specs, check_rep=False),
                donate_argnums=donate, keep_unused=True)

        self.out_avals = out_avals
        zsh = NamedSharding(mesh, PartitionSpec("core"))
        zspecs = [((NCORES * z.shape[0], *z.shape[1:]), z.dtype)
                  for z in zero_outs]
        # AOT-compile with the bass effect suppressed -> jax C++ fast-path
        # dispatch (~1ms less python overhead before the request hits the
        # wire). Fall back to the plain jit if unavailable.
        try:
            in_sds = [jax.ShapeDtypeStruct((NCORES * s[0], *s[1:]), d,
                                           sharding=zsh)
                      for (s, d) in in_shapes]
            z_sds = [jax.ShapeDtypeStruct(s, d, sharding=zsh)
                     for (s, d) in zspecs]
            self.fn = b2j.fast_dispatch_compile(
                lambda: _mk_jit().lower(*in_sds, *z_sds).compile())
        except Exception:
            self.fn = _mk_jit()
        # donated zero output buffers, created on-device (no H2D)
        self.zfn = jax.jit(
            lambda: tuple(jnp.zeros(s, d) for s, d in zspecs),
            out_shardings=tuple(zsh for _ in zspecs))
        self.in_sharding = zsh
        self._zcache = None

    def __call__(self, per_core_maps):
        concat_in = [np.concatenate([np.asarray(per_core_maps[c][nm])
                                     for c in range(NCORES)], axis=0)
                     for nm in self.in_names]
        return self.run_concat(concat_in)

    def run_raw(self, concat_in, zeros=None):
        """concat_in: list of [NCORES*s0, ...] arrays (np or device jax).
        Returns tuple of sharded jax output arrays. The donated output
        buffers come from a one-slot cache (stash_outputs recycles the
        previous call's fully-overwritten outputs), else a zeros jit."""
        concat_in = [x if hasattr(x, "addressable_shards")
                     else self.jax.device_put(np.ascontiguousarray(x),
                                              self.in_sharding)
                     for x in concat_in]
        if zeros is None:
            zeros, self._zcache = self._zcache, None
            if zeros is None:
                zeros = self.zfn()
        return self.fn(*concat_in, *zeros)

    def stash_outputs(self, raw):
        """Recycle output arrays as the next call's donated buffers (valid
        because every output byte is rewritten on device each run)."""
        self._zcache = tuple(raw)

    def run_concat(self, concat_in):
        out = self.run_raw(concat_in)
        res = []
        for c in range(NCORES):
            res.append({nm: np.asarray(out[i]).reshape(
                NCORES, *self.out_avals[i].shape)[c]
                for i, nm in enumerate(self.out_names)})
        return res


def _get_runner(name):
    if name not in _PROGS:
        build = {"l1": _build_l1, "l2": _build_l2, "l3": _build_l3,
                 "l23": _build_l23, "l123": _build_l123}[name]
        nc, ins, outs = build()
        _PROGS[name] = _Runner(nc)
    return _PROGS[name]


# --------------------------------------------------- on-device greedy (jax)
def _greedy_core_jax(klp, valid_row):
    """Replicates _klmat_host + _greedy decision-for-decision in f32.

    klp: [257,256] f32 (rows 0..255 = C, row 256 = sx), valid_row [1,256] f32.
    Returns compacted sel [256,256] f32, cnt [256] f32, newvalid [256] bool.
    """
    import jax.numpy as jnp
    from jax import lax
    C = klp[0:256]
    sx = klp[256]
    t = sx[:, None] + sx[None, :]
    t = t - C
    t = t - C.T
    kl = jnp.float32(0.5) * t
    valid = valid_row[0] > jnp.float32(0.5)
    kl = jnp.where(valid[None, :], kl, jnp.float32(np.inf))
    adj = (kl < jnp.float32(0.9)) & valid[None, :]

    def body(i, carry):
        matched, active = carry
        a = valid[i] & (~matched[i])
        matched = jnp.where(a, matched | adj[i], matched)
        active = active.at[i].set(a)
        return matched, active

    matched0 = jnp.zeros((256,), bool)
    active0 = jnp.zeros((256,), bool)
    _, active = lax.fori_loop(0, 256, body, (matched0, active0))
    ranks = jnp.cumsum(active.astype(jnp.int32)) - 1
    ocount = jnp.sum(active.astype(jnp.int32))
    rows = adj.astype(jnp.float32) * active[:, None].astype(jnp.float32)
    idx = jnp.where(active, ranks, 256)
    sel = jnp.zeros((257, 256), jnp.float32).at[idx].add(rows)[0:256]
    cnt = jnp.maximum(jnp.sum(sel, 1), jnp.float32(1.0))
    newvalid = jnp.arange(256) < ocount
    return sel, cnt, newvalid


def _get_greedy_jits():
    if "g2" not in _PROGS:
        import jax
        import jax.numpy as jnp
        from jax.sharding import Mesh, PartitionSpec
        from jax.experimental.shard_map import shard_map
        mesh = Mesh(np.asarray(jax.devices()[:NCORES]), ("core",))
        P = PartitionSpec

        def g2_body(klp):
            sel, cnt, newvalid = _greedy_core_jax(
                klp, jnp.ones((1, 256), jnp.float32))
            vrow = newvalid.astype(jnp.float32)[None, :]
            return (sel.T.astype(jnp.uint8), cnt[:, None],
                    vrow, jnp.float32(1.0) - vrow)

        def g3_body(klp, vrow2):
            sel, cnt, newvalid = _greedy_core_jax(klp, vrow2)
            biasv = jnp.where(newvalid, jnp.float32(0.0),
                              NEG_BIG)[:, None].astype(jnp.float32)
            return sel.T.astype(jnp.uint8), cnt[:, None], biasv

        _PROGS["g2"] = jax.jit(shard_map(
            g2_body, mesh=mesh, in_specs=(P("core"),),
            out_specs=(P("core"),) * 4, check_rep=False))
        _PROGS["g3"] = jax.jit(shard_map(
            g3_body, mesh=mesh, in_specs=(P("core"), P("core")),
            out_specs=(P("core"),) * 3, check_rep=False))
    return _PROGS["g2"], _PROGS["g3"]


_SMALL_DEV = {}


def _small_const_dev():
    """Per-call constant small inputs, staged on device once."""
    if not _SMALL_DEV:
        import jax
        from jax.sharding import Mesh, PartitionSpec, NamedSharding
        mesh = Mesh(np.asarray(jax.devices()[:NCORES]), ("core",))
        sh = NamedSharding(mesh, PartitionSpec("core"))
        _SMALL_DEV["iwin"] = jax.device_put(_IWIN_CAT, sh)
    return _SMALL_DEV


# ------------------------------------------------------------------- host math
def _greedy(klmat, valid):
    """Reference greedy loop via 256-bit ints. Returns sel bool [256,256], oc."""
    N = klmat.shape[0]
    adj = (klmat < np.float32(0.9)) & valid[None, :]
    rows = [int.from_bytes(np.packbits(adj[i], bitorder='little').tobytes(),
                           'little') for i in range(N)]
    vbits = int.from_bytes(np.packbits(valid, bitorder='little').tobytes(),
                           'little')
    matched = 0
    sel_rows = []
    for i in range(N):
        if (vbits >> i) & 1 and not (matched >> i) & 1:
            matched |= rows[i]
            sel_rows.append(rows[i])
    sel = np.zeros((N, N), bool)
    for o, r in enumerate(sel_rows):
        sel[o] = np.unpackbits(
            np.frombuffer(r.to_bytes(32, 'little'), np.uint8),
            bitorder='little')[:N]
    return sel, len(sel_rows)


def _klmat_host(sx, C):
    """0.5*(((sx_i+sx_j) - C) - C.T) in f32, matching the reference order."""
    t = (sx[:, None] + sx[None, :]).astype(np.float32)
    t = t - C
    t = t - C.T
    return (np.float32(0.5) * t).astype(np.float32)


def _prep_l1_inputs(w64, w32, w16, w8):
    cat64 = np.empty((64, 512, 4096), np.float32)
    cat32 = np.empty((64, 256, 1024), np.float32)
    cat16 = np.empty((64, 128, 256), np.float32)
    cat8 = np.empty((64, 64, 64), np.float32)
    for k in range(NCORES):
        cat64[8 * k:8 * k + 8] = w64[:, 512 * k:512 * k + 512, :]
        r32 = (8 * k) % 32 * 32
        cat32[8 * k:8 * k + 8] = w32[:, r32:r32 + 256, :]
        r16 = (8 * k) % 16 * 16
        cat16[8 * k:8 * k + 8] = w16[:, r16:r16 + 128, :]
        cat8[8 * k:8 * k + 8] = w8
    return [cat64, cat32, cat16, cat8]


def _segment_one(w64, w32, w16, w8, l1_dev_in=None):
    r = _get_runner("l123")
    wins = (l1_dev_in if l1_dev_in is not None
            else _prep_l1_inputs(w64, w32, w16, w8))
    per = {"w64s": wins[0], "w32s": wins[1], "w16s": wins[2], "w8s": wins[3],
           "iwin": _small_const_dev()["iwin"]}
    per.update(_l3_const_dev())
    raw = r.run_raw([per[nm] for nm in r.in_names])
    # single blocking host sync for the whole chain: fetch core 0's shard.
    # pack: [8,128,66] u8 = labels (scattered anchor idx, cols 0..63) +
    # the greedy-3 active mask a3 (cols 64/65).
    pack = np.asarray(
        raw[r.out_names.index("opack")].addressable_shards[0].data)
    r.stash_outputs(raw)   # recycle buffers: no zero-fill dispatch next call
    a3 = np.concatenate([pack[0][:, 64], pack[0][:, 65]]).astype(np.int32)
    lut = (np.cumsum(a3) - 1).astype(np.int32)   # scattered idx -> compact
    out = lut[pack[:, :, 0:64].transpose(0, 2, 1).reshape(-1)]
    return out.reshape(256, 256)


def kernel(**inputs):
    w64 = np.asarray(inputs["weight_64"], np.float32)
    w32 = np.asarray(inputs["weight_32"], np.float32)
    w16 = np.asarray(inputs["weight_16"], np.float32)
    w8 = np.asarray(inputs["weight_8"], np.float32)
    B = w64.shape[0]
    outs = [_segment_one(w64[b], w32[b], w16[b], w8[b]) for b in range(B)]
    return np.stack(outs).astype(np.int32)



# revision 22
# speedup vs baseline: 1.0062x; 1.0062x over previous
"""DiffSeg segmentation head on 8 Trainium2 NeuronCores (Bass/Tile).

ONE bass program (l123), ONE dispatch, ONE blocking host fetch per image:
  L1 section: multi-scale aggregation of attention maps -> agg [4096,4096]
      f32 (row-sharded 512 rows/core), anchor gather + symmetric-KL merge0
      (fp16 log/matmuls like the reference), ReduceScatter -> X2 p-slices,
      C1 = X2 @ lX2.T + sx packed into one [257,256] AllReduce.
  greedy 2 (on-device, replicated per core): klmat assembled from the
      AllReduced pack with the reference's exact f32 op order; the
      sequential 256-step greedy selection runs as an unrolled loop
      (PE one-hot row extraction + 3 DVE ops per step) in SCATTERED form:
      rows stay at their anchor index, an active mask a2[] marks
      representatives (compaction is order-preserving so all downstream
      values are identical).
  L2 section: selT = adjacency*mask, new2 = sel @ X2 / cnt (PE), AllGather,
      window gather, masked ln, C3/sx3 pack -> [257,256] AllReduce.
  greedy 3: same unrolled loop with valid = a2 -> a3.
  L3 section: new3 = sel3 @ new2_window / cnt + (-1e38 inactive bias),
      4x bilinear upsample (align_corners), transpose, per-pixel argmax,
      labels packed as u8 together with a3, AllGather -> single output.
  host: one fetch of the [8,128,66] u8 pack; scattered argmax indices are
      remapped to the reference's compacted labels via a cumsum LUT on a3.

Every host round-trip over the axon tunnel costs ~30-90ms (ambient), so
the whole computation is a single async dispatch + a single fetch; the
on-device pipeline itself adds only ~4ms.

Self-contained: hardcodes shapes/sharding for inputs
  weight_64 [B,8,4096,4096], weight_32 [B,8,1024,1024],
  weight_16 [B,8,256,256],   weight_8  [B,8,64,64]  (B=1)
"""

import sys
import numpy as np

for _p in ("/opt/trn_rl_repo", "/opt/trn_rl_repo/concourse"):
    if _p not in sys.path:
        sys.path.append(_p)

NCORES = 8
THR2 = np.float32(1.8)          # == 2 * f32(0.9), exact
RAT = [np.float32(64.0 / 120.0), np.float32(32.0 / 120.0),
       np.float32(16.0 / 120.0), np.float32(8.0 / 120.0)]
NEG_BIG = np.float32(-1e38)


def _up_coords(n, r):
    s = np.linspace(0.0, n - 1.0, n * r)
    i0 = np.clip(np.floor(s).astype(np.int64), 0, n - 2)
    w = (s - i0).astype(np.float32)
    return i0.astype(np.int64), w


# ---------------------------------------------------------------- host consts
X032, W32 = _up_coords(32, 2)      # 32 -> 64
X016, W16 = _up_coords(16, 4)
X08, W8 = _up_coords(8, 8)
Y0U, WYU = _up_coords(64, 4)       # 64 -> 256 (final upsample)
X0U, WXU = _up_coords(64, 4)

# L3 per-core y windows
L3_LO = [int(Y0U[32 * k]) for k in range(NCORES)]          # window start row
L3_W = 12  # padded window size (12 = smallest mult-of-4 >= 10)

ANCHOR_PTS = [1 + 4 * i for i in range(16)]


def _wrap_idx(vals):
    """ap_gather index layout: j -> partition j%16, elem j//16; replicated to
    all 8 gpsimd cores (16-partition groups)."""
    n = len(vals)
    cols = (n + 15) // 16
    arr = np.zeros((16, cols), np.int16)
    for j, v in enumerate(vals):
        arr[j % 16, j // 16] = v
    return np.tile(arr, (8, 1))


def _l3_host_consts():
    consts = []
    # x-gather indices (uniform across cores): view [32y*64x]; idx = y*64+x0(j)
    idxx_c = np.array([y * 64 + X0U[j] for y in range(32) for j in range(256)],
                      np.int16)
    idxx_d = np.array([y * 64 + X0U[j] + 1 for y in range(32) for j in range(256)],
                      np.int16)
    wxr = np.broadcast_to(WXU[None, :], (32, 256)).reshape(1, -1).astype(np.float32)
    for k in range(NCORES):
        lo = L3_LO[k]
        y0l = [int(Y0U[32 * k + t]) - lo for t in range(32)]
        consts.append(dict(
            idxyc=_wrap_idx(np.array(y0l, np.int16)),
            idxyd=_wrap_idx(np.array(y0l, np.int16) + 1),
            wy=WYU[32 * k:32 * k + 32][None, :].astype(np.float32),
            idxxc=_wrap_idx(idxx_c),
            idxxd=_wrap_idx(idxx_d),
            wx=WXU[None, :].astype(np.float32),
        ))
    return consts


L3C = _l3_host_consts()
_IWIN_CAT = np.concatenate(
    [_wrap_idx(np.clip(np.arange(L3_LO[k], L3_LO[k] + L3_W), 0, 63)
               .astype(np.int16)) for k in range(NCORES)], axis=0)
_L3_DEV = {}


def _l3_const_dev():
    """Constant L3 inputs, concatenated and staged on device once."""
    if not _L3_DEV:
        import jax
        from jax.sharding import Mesh, PartitionSpec, NamedSharding
        mesh = Mesh(np.asarray(jax.devices()[:NCORES]), ("core",))
        sh = NamedSharding(mesh, PartitionSpec("core"))
        for nm in ("wy", "wx", "idxyc", "idxyd", "idxxc", "idxxd"):
            cat = np.concatenate([L3C[k][nm] for k in range(NCORES)], axis=0)
            _L3_DEV[nm] = jax.device_put(cat, sh)
    return _L3_DEV

# ------------------------------------------------------------------- programs
_PROGS = {}


def _mybir():
    from concourse import mybir
    return mybir


def _build_common():
    import concourse.tile as tile
    from concourse import mybir, bacc
    from concourse.bass_utils import axon_active
    nc = bacc.Bacc("TRN2", target_bir_lowering=False, debug=False,
                   enable_asserts=False, num_devices=NCORES)
    return nc, tile, mybir



def _emit_l1(nc, tile, mybir, tc, w64, w32, w16, w8, o_x2, o_kl):
    from concourse.masks import make_identity
    F32, F16 = mybir.dt.float32, mybir.dt.float16
    ALU, AX, AF = mybir.AluOpType, mybir.AxisListType, mybir.ActivationFunctionType
    RG = [list(range(NCORES))]
    if True:
        with tc.tile_pool(name="consts", bufs=1) as cpool, \
             tc.tile_pool(name="ystore", bufs=1) as ystore, \
             tc.tile_pool(name="dram", bufs=1, space="DRAM") as dram:

            ident16 = cpool.tile([128, 128], F16, tag="id16", name="id16")
            make_identity(nc, ident16[:])
            ident32 = cpool.tile([128, 128], F32, tag="id32", name="id32")
            make_identity(nc, ident32[:])
            ones16 = cpool.tile([128, 1], F16, tag="o16", name="o16")
            nc.gpsimd.memset(ones16[:], 1.0)
            ones32 = cpool.tile([128, 1], F32, tag="o32", name="o32")
            nc.gpsimd.memset(ones32[:], 1.0)

            Y = [ystore.tile([128, 4096], F32, tag=f"Y{pt}", name=f"Y{pt}")
                 for pt in range(4)]

            # ============ phases A+B: aggregation ============
            with tc.tile_pool(name="nmap", bufs=1) as nmap, \
                 tc.tile_pool(name="stage", bufs=2) as stage, \
                 tc.tile_pool(name="workab", bufs=1) as workab:

                def upsample_norm(src_dram, s, P, tidx, out_tile):
                    """head-sum -> bilinear s->64 (x then y) -> normalize."""
                    ssq = s * s
                    m = workab.tile([P, ssq], F32, tag="mA", name="mA")
                    ncb = max(1, ssq // 256)
                    cw = ssq // ncb
                    for cb in range(ncb):
                        stg = stage.tile([P, 8, cw], F32, tag="stg", name="stg")
                        for h in range(8):
                            nc.sync.dma_start(
                                stg[:, h, :],
                                src_dram[h, tidx * P:(tidx + 1) * P,
                                         cb * cw:(cb + 1) * cw])
                        nc.vector.tensor_reduce(
                            m[:, cb * cw:(cb + 1) * cw],
                            stg[:].rearrange("p h x -> p x h"), AX.X, ALU.add)
                    m3 = m[:].rearrange("p (y x) -> p y x", y=s)
                    x0s, wxs = {32: (X032, W32), 16: (X016, W16),
                                8: (X08, W8)}[s]
                    ux = workab.tile([P, s, 64], F32, tag="uxA", name="uxA")
                    tmp = workab.tile([P, s], F32, tag="tmpxA", name="tmpxA")
                    for j in range(64):
                        c = m3[:, :, int(x0s[j])]
                        d = m3[:, :, int(x0s[j]) + 1]
                        eng = nc.vector if j % 2 == 0 else nc.gpsimd
                        eng.tensor_tensor(tmp[:], d, c, ALU.subtract)
                        nc.vector.scalar_tensor_tensor(
                            ux[:, :, j], tmp[:], float(wxs[j]), c,
                            ALU.mult, ALU.add)
                    tmp2 = workab.tile([P, 64], F32, tag="tmpyA", name="tmpyA")
                    for j in range(64):
                        c = ux[:, int(x0s[j]), :]
                        d = ux[:, int(x0s[j]) + 1, :]
                        eng = nc.vector if j % 2 == 0 else nc.gpsimd
                        eng.tensor_tensor(tmp2[:], d, c, ALU.subtract)
                        nc.vector.scalar_tensor_tensor(
                            out_tile[:, j, :], tmp2[:], float(wxs[j]), c,
                            ALU.mult, ALU.add)
                    rs = workab.tile([P, 1], F32, tag="rsA", name="rsA")
                    nc.vector.tensor_reduce(rs[:], out_tile[:], AX.XY, ALU.add)
                    nc.vector.reciprocal(rs[:], rs[:])
                    flat = out_tile[:].rearrange("p a b -> p (a b)")
                    nc.vector.tensor_scalar(flat, flat, rs[:], None, ALU.mult)

                n32 = [nmap.tile([128, 64, 64], F32, tag=f"n32_{t}",
                                 name=f"n32_{t}") for t in range(2)]
                for t in range(2):
                    upsample_norm(w32, 32, 128, t, n32[t])
                n16 = nmap.tile([128, 64, 64], F32, tag="n16", name="n16")
                upsample_norm(w16, 16, 128, 0, n16)
                n8 = nmap.tile([64, 64, 64], F32, tag="n8", name="n8")
                upsample_norm(w8, 8, 64, 0, n8)

                for pt in range(4):
                    for cb in range(8):
                        stg = stage.tile([128, 8, 512], F32, tag="stg",
                                         name="stg64")
                        for h in range(8):
                            nc.sync.dma_start(
                                stg[:, h, :],
                                w64[h, pt * 128:(pt + 1) * 128,
                                    cb * 512:(cb + 1) * 512])
                        nc.vector.tensor_reduce(
                            Y[pt][:, cb * 512:(cb + 1) * 512],
                            stg[:].rearrange("p h x -> p x h"), AX.X, ALU.add)
                    rs = workab.tile([128, 1], F32, tag="rsY", name="rsY")
                    nc.vector.tensor_reduce(rs[:], Y[pt][:], AX.X, ALU.add)
                    nc.vector.reciprocal(rs[:], rs[:])
                    nc.vector.tensor_scalar(rs[:], rs[:], float(RAT[0]), None,
                                            ALU.mult)
                    nc.vector.tensor_scalar(Y[pt][:], Y[pt][:], rs[:], None,
                                            ALU.mult)
                    rep = workab.tile([128, 4096], F32, tag="rep", name="rep")
                    srct = n32[pt // 2]
                    base = (2 * pt) % 4 * 32
                    for ar in range(2):
                        for rp in range(2):
                            nc.sync.dma_start(
                                rep[ar * 64 + rp * 32:ar * 64 + rp * 32 + 32, :],
                                srct[base + ar * 32:base + ar * 32 + 32, :, :]
                                .rearrange("p a b -> p (a b)"))
                    nc.vector.scalar_tensor_tensor(
                        Y[pt][:], rep[:], float(RAT[1]), Y[pt][:],
                        ALU.mult, ALU.add)
                    rep2 = workab.tile([128, 4096], F32, tag="rep", name="rep2")
                    for ar in range(2):
                        a_loc = 2 * pt + ar
                        for rp in range(4):
                            nc.sync.dma_start(
                                rep2[ar * 64 + rp * 16:ar * 64 + rp * 16 + 16, :],
                                n16[a_loc * 16:a_loc * 16 + 16, :, :]
                                .rearrange("p a b -> p (a b)"))
                    nc.vector.scalar_tensor_tensor(
                        Y[pt][:], rep2[:], float(RAT[2]), Y[pt][:],
                        ALU.mult, ALU.add)
                    rep3 = workab.tile([128, 4096], F32, tag="rep", name="rep3")
                    for ar in range(2):
                        a_loc = 2 * pt + ar
                        for rp in range(8):
                            nc.sync.dma_start(
                                rep3[ar * 64 + rp * 8:ar * 64 + rp * 8 + 8, :],
                                n8[(a_loc % 8) * 8:(a_loc % 8) * 8 + 8, :, :]
                                .rearrange("p a b -> p (a b)"))
                    nc.vector.scalar_tensor_tensor(
                        Y[pt][:], rep3[:], float(RAT[3]), Y[pt][:],
                        ALU.mult, ALU.add)

            # ============ phase C: merge0 ============
            with tc.tile_pool(name="xstore", bufs=1) as xstore, \
                 tc.tile_pool(name="workc", bufs=2) as workc:
                # natural-layout fp16 + sy columns
                yh_nat = [xstore.tile([128, 4096], F16, tag=f"yhn{pt}",
                                      name=f"yhn{pt}") for pt in range(4)]
                sycol = [cpool.tile([128, 1], F32, tag=f"syc{mt}",
                                    name=f"syc{mt}") for mt in range(4)]
                for pt in range(4):
                    nc.scalar.activation(yh_nat[pt][:], Y[pt][:], AF.Copy)
                    lnn = workc.tile([128, 4096], F16, tag="lnn", name="lnn",
                                     bufs=1)
                    nc.scalar.activation(lnn[:], yh_nat[pt][:], AF.Ln)
                    nc.vector.tensor_tensor(lnn[:], yh_nat[pt][:], lnn[:],
                                            ALU.mult)
                    nc.vector.tensor_reduce(sycol[pt][:], lnn[:], AX.X,
                                            ALU.add)

                # anchors -> allgather -> X^T, lX^T
                psA_cm = tc.tile_pool(name="psA", bufs=2, space="PSUM")
                psA = psA_cm.__enter__()
                psAs_cm = tc.tile_pool(name="psAs", bufs=1, space="PSUM")
                psAs = psAs_cm.__enter__()
                xloc = workc.tile([32, 4096], F32, tag="xloc", name="xloc",
                                  bufs=1)
                nc.sync.dma_start(xloc[0:16, :], Y[0][65:126:4, :])
                nc.sync.dma_start(xloc[16:32, :], Y[2][65:126:4, :])
                xloch = workc.tile([32, 4096], F16, tag="xloch", name="xloch",
                                   bufs=1)
                nc.scalar.activation(xloch[:], xloc[:], AF.Copy)
                bx_in = dram.tile([32, 4096], F16, name="bx_in")
                bx_out = dram.tile([8, 32, 4096], F16, name="bx_out")
                nc.sync.dma_start(bx_in[:], xloch[:])
                nc.gpsimd.collective_compute(
                    "AllGather", ALU.bypass, replica_groups=RG,
                    ins=[bx_in.opt()], outs=[bx_out.opt()])
                xT = xstore.tile([128, 32, 256], F16, tag="xT", name="xT")
                lxT = xstore.tile([128, 32, 256], F16, tag="lxT", name="lxT")
                bxv = bx_out[:].rearrange("c a p -> (c a) p")
                for nt in range(2):
                    xnat = workc.tile([128, 4096], F16, tag="xnat",
                                      name="xnat", bufs=1)
                    nc.sync.dma_start(xnat[:], bxv[nt * 128:(nt + 1) * 128, :])
                    for ct in range(32):
                        pst = psA.tile([128, 128], F16, tag="tp16",
                                       name="tp16")
                        nc.tensor.transpose(
                            pst[:], xnat[:, ct * 128:(ct + 1) * 128],
                            ident16[:])
                        nc.any.tensor_copy(
                            xT[:, ct, nt * 128:(nt + 1) * 128], pst[:])
                for ct in range(32):
                    nc.scalar.activation(lxT[:, ct, :], xT[:, ct, :], AF.Ln)
                sxP = psAs.tile([1, 256], F32, tag="sx", name="sxP")
                for ct in range(32):
                    prodx = workc.tile([128, 256], F16, tag="prodX",
                                       name="prodX")
                    nc.vector.tensor_tensor(prodx[:], xT[:, ct, :],
                                            lxT[:, ct, :], ALU.mult)
                    nc.tensor.matmul(sxP[:], ones16[:], prodx[:],
                                     start=(ct == 0), stop=(ct == 31))
                sx_sb = workc.tile([1, 256], F32, tag="sxsb", name="sxsb",
                                   bufs=1)
                nc.any.tensor_copy(sx_sb[:], sxP[:])
                sxb = cpool.tile([128, 256], F32, tag="sxb", name="sxb")
                nc.gpsimd.partition_broadcast(sxb[:], sx_sb[:])

                # cross accumulation with rolling transposed ct-tiles
                psC = [psA.tile([128, 256], F32, tag=f"psC{mt}",
                                name=f"psC{mt}", bufs=1) for mt in range(4)]
                for ct in range(32):
                    yhTct = workc.tile([128, 512], F16, tag="yhTct",
                                       name="yhTct")
                    for pt in range(4):
                        pst = psA.tile([128, 128], F16, tag="tp16",
                                       name="tp16b")
                        nc.tensor.transpose(
                            pst[:], yh_nat[pt][:, ct * 128:(ct + 1) * 128],
                            ident16[:])
                        nc.any.tensor_copy(
                            yhTct[:, pt * 128:(pt + 1) * 128], pst[:])
                    lnct = workc.tile([128, 512], F16, tag="lnct", name="lnct")
                    nc.scalar.activation(lnct[:], yhTct[:], AF.Ln)
                    for mt in range(4):
                        nc.tensor.matmul(
                            psC[mt][:], lnct[:, mt * 128:(mt + 1) * 128],
                            xT[:, ct, :], start=(ct == 0), stop=False)
                        nc.tensor.matmul(
                            psC[mt][:], yhTct[:, mt * 128:(mt + 1) * 128],
                            lxT[:, ct, :], start=False, stop=(ct == 31))
                knT = xstore.tile([128, 4, 256], F32, tag="knT", name="knT")
                for mt in range(4):
                    S = workc.tile([128, 256], F32, tag="Ssum", name="Ssum")
                    nc.vector.tensor_scalar(S[:], sxb[:], sycol[mt][:], None,
                                            ALU.add)
                    nc.vector.tensor_tensor(S[:], S[:], psC[mt][:],
                                            ALU.subtract)
                    nc.vector.tensor_scalar(knT[:, mt, :], S[:], float(THR2),
                                            None, ALU.is_lt)

                psAs_cm.__exit__(None, None, None)
                psA_cm.__exit__(None, None, None)
                # new^T partials, counts, collectives
                psB_cm = tc.tile_pool(name="psB", bufs=2, space="PSUM")
                psB = psB_cm.__enter__()
                psBs_cm = tc.tile_pool(name="psBs", bufs=1, space="PSUM")
                psBs = psBs_cm.__enter__()
                cntP = psBs.tile([1, 256], F32, tag="cnt", name="cntP")
                for mt in range(4):
                    nc.tensor.matmul(cntP[:], ones32[:], knT[:, mt, :],
                                     start=(mt == 0), stop=(mt == 3))
                bN_in = dram.tile([4096, 256], F32, name="bN_in")
                for ptile in range(32):
                    psN = psB.tile([128, 256], F32, tag="psN", name="psN")
                    for mt in range(4):
                        nc.tensor.matmul(
                            psN[:], Y[mt][:, ptile * 128:(ptile + 1) * 128],
                            knT[:, mt, :], start=(mt == 0), stop=(mt == 3))
                    npt = workc.tile([128, 256], F32, tag="npt", name="npt")
                    nc.any.tensor_copy(npt[:], psN[:])
                    nc.sync.dma_start(bN_in[ptile * 128:(ptile + 1) * 128, :],
                                      npt[:])
                bC_in = dram.tile([1, 256], F32, name="bC_in")
                bC_out = dram.tile([1, 256], F32, name="bC_out")
                cnt_sb = workc.tile([1, 256], F32, tag="cntsb", name="cntsb",
                                    bufs=1)
                nc.any.tensor_copy(cnt_sb[:], cntP[:])
                nc.sync.dma_start(bC_in[:], cnt_sb[:])
                nc.gpsimd.collective_compute(
                    "AllReduce", ALU.add, replica_groups=RG,
                    ins=[bC_in.opt()], outs=[bC_out.opt()])
                bN_out = dram.tile([512, 256], F32, name="bN_out")
                nc.gpsimd.collective_compute(
                    "ReduceScatter", ALU.add, replica_groups=RG,
                    ins=[bN_in.opt()], outs=[bN_out.opt()])

                # X2^T, lX2^T, sx2 partial, C1 partial, outputs
                cntg = workc.tile([1, 256], F32, tag="cntg", name="cntg",
                                  bufs=1)
                nc.sync.dma_start(cntg[:], bC_out[:])
                nc.vector.reciprocal(cntg[:], cntg[:])
                cb2 = cpool.tile([128, 256], F32, tag="cb2", name="cb2")
                nc.gpsimd.partition_broadcast(cb2[:], cntg[:])
                x2T = [xstore.tile([128, 256], F32, tag=f"x2T{t}",
                                   name=f"x2T{t}") for t in range(4)]
                lx2T = [xstore.tile([128, 256], F32, tag=f"lx2T{t}",
                                    name=f"lx2T{t}") for t in range(4)]
                for t in range(4):
                    nc.sync.dma_start(x2T[t][:],
                                      bN_out[t * 128:(t + 1) * 128, :])
                    nc.vector.tensor_tensor(x2T[t][:], x2T[t][:], cb2[:],
                                            ALU.mult)
                    nc.scalar.activation(lx2T[t][:], x2T[t][:], AF.Ln)
                sx2P = psBs.tile([1, 256], F32, tag="sx2", name="sx2P")
                for t in range(4):
                    prod2 = workc.tile([128, 256], F32, tag="prod2",
                                       name="prod2")
                    nc.vector.tensor_tensor(prod2[:], x2T[t][:], lx2T[t][:],
                                            ALU.mult)
                    nc.tensor.matmul(sx2P[:], ones32[:], prod2[:],
                                     start=(t == 0), stop=(t == 3))
                sx2sb = workc.tile([1, 256], F32, tag="sx2sb", name="sx2sb",
                                   bufs=1)
                nc.any.tensor_copy(sx2sb[:], sx2P[:])
                bS_in = dram.tile([1, 256], F32, name="bS_in")
                bS_out = dram.tile([1, 256], F32, name="bS_out")
                nc.sync.dma_start(bS_in[:], sx2sb[:])
                nc.gpsimd.collective_compute(
                    "AllReduce", ALU.add, replica_groups=RG,
                    ins=[bS_in.opt()], outs=[bS_out.opt()])
                nc.sync.dma_start(o_kl[256:257, :], bS_out[:])
                bC1_in = dram.tile([256, 256], F32, name="bC1_in")
                bC1_out = dram.tile([256, 256], F32, name="bC1_out")
                for it in range(2):
                    psC1 = psB.tile([128, 256], F32, tag="pc1", name="pc1")
                    for kt in range(4):
                        nc.tensor.matmul(
                            psC1[:], x2T[kt][:, it * 128:(it + 1) * 128],
                            lx2T[kt][:], start=(kt == 0), stop=(kt == 3))
                    c1t = workc.tile([128, 256], F32, tag="c1t", name="c1t")
                    nc.any.tensor_copy(c1t[:], psC1[:])
                    nc.sync.dma_start(bC1_in[it * 128:(it + 1) * 128, :],
                                      c1t[:])
                nc.gpsimd.collective_compute(
                    "AllReduce", ALU.add, replica_groups=RG,
                    ins=[bC1_in.opt()], outs=[bC1_out.opt()])
                nc.sync.dma_start(o_kl[0:256, :], bC1_out[:])
                for it in range(2):
                    x2n = workc.tile([128, 512], F32, tag="x2n", name="x2n")
                    for kt in range(4):
                        pst = psB.tile([128, 128], F32, tag="tp32",
                                       name="tp32")
                        nc.tensor.transpose(
                            pst[:], x2T[kt][:, it * 128:(it + 1) * 128],
                            ident32[:])
                        nc.any.tensor_copy(x2n[:, kt * 128:(kt + 1) * 128],
                                           pst[:])
                    nc.sync.dma_start(o_x2[it * 128:(it + 1) * 128, :],
                                      x2n[:])
                psBs_cm.__exit__(None, None, None)
                psB_cm.__exit__(None, None, None)


def _build_l1():
    nc, tile, mybir = _build_common()
    F32 = mybir.dt.float32
    w64 = nc.dram_tensor("w64s", [8, 512, 4096], F32, kind="ExternalInput")
    w32 = nc.dram_tensor("w32s", [8, 256, 1024], F32, kind="ExternalInput")
    w16 = nc.dram_tensor("w16s", [8, 128, 256], F32, kind="ExternalInput")
    w8 = nc.dram_tensor("w8s", [8, 64, 64], F32, kind="ExternalInput")
    o_x2 = nc.dram_tensor("x2slice", [256, 512], F32, kind="ExternalOutput")
    o_kl = nc.dram_tensor("klpack", [257, 256], F32, kind="ExternalOutput")
    with tile.TileContext(nc) as tc:
        _emit_l1(nc, tile, mybir, tc, w64, w32, w16, w8, o_x2, o_kl)
    nc.finalize()
    return nc, ["w64s", "w32s", "w16s", "w8s"], ["x2slice", "klpack"]



def _build_l2():
    nc, tile, mybir = _build_common()
    from concourse.masks import make_identity
    F32 = mybir.dt.float32
    ALU, AX, AF = mybir.AluOpType, mybir.AxisListType, mybir.ActivationFunctionType

    x2s = nc.dram_tensor("x2s", [256, 512], F32, kind="ExternalInput")
    selT = nc.dram_tensor("sel2T", [256, 256], mybir.dt.uint8,
                          kind="ExternalInput")
    icnt = nc.dram_tensor("icnt2", [256, 1], F32, kind="ExternalInput")
    vrow = nc.dram_tensor("vrow", [1, 256], F32, kind="ExternalInput")
    irow = nc.dram_tensor("irow", [1, 256], F32, kind="ExternalInput")
    iwin = nc.dram_tensor("iwin", [128, 1], mybir.dt.int16,
                          kind="ExternalInput")
    o_n2w = nc.dram_tensor("n2w", [256, L3_W * 64], F32,
                           kind="ExternalOutput")
    o_kl3 = nc.dram_tensor("klpack3", [257, 256], F32, kind="ExternalOutput")
    RG = [list(range(NCORES))]

    with tile.TileContext(nc) as tc:
        with tc.tile_pool(name="sb", bufs=1) as pool, \
             tc.tile_pool(name="work", bufs=2) as work, \
             tc.tile_pool(name="psum", bufs=2, space="PSUM") as psum, \
             tc.tile_pool(name="psumS", bufs=1, space="PSUM") as psumS, \
             tc.tile_pool(name="dram", bufs=1, space="DRAM") as dram:
            ident32 = pool.tile([128, 128], F32, tag="id32", name="id32")
            make_identity(nc, ident32[:])
            ones32 = pool.tile([128, 1], F32, tag="o32", name="o32")
            nc.gpsimd.memset(ones32[:], 1.0)
            iw = pool.tile([128, 1], mybir.dt.int16, tag="iw", name="iw")
            nc.sync.dma_start(iw[:], iwin[:])
            xs = [pool.tile([128, 512], F32, tag=f"xs{t}", name=f"xs{t}") for t in range(2)]
            st = [pool.tile([128, 256], F32, tag=f"st{t}", name=f"st{t}") for t in range(2)]
            stu = [pool.tile([128, 256], mybir.dt.uint8, tag=f"stu{t}",
                             name=f"stu{t}") for t in range(2)]
            for t in range(2):
                nc.sync.dma_start(xs[t][:], x2s[t * 128:(t + 1) * 128, :])
                nc.sync.dma_start(stu[t][:], selT[t * 128:(t + 1) * 128, :])
                nc.any.tensor_copy(st[t][:], stu[t][:])
            cnt = pool.tile([128, 2], F32, tag="cnt", name="cnt")
            nc.sync.dma_start(cnt[:], icnt[:].rearrange("(a p) b -> p (a b)", a=2))
            rc = pool.tile([128, 2], F32, tag="rc", name="rc")
            nc.vector.reciprocal(rc[:], cnt[:])
            vb = pool.tile([128, 256], F32, tag="vb", name="vb")
            ib = pool.tile([128, 256], F32, tag="ib", name="ib")
            vsb = work.tile([1, 256], F32, tag="vsb", name="vsb")
            isb = work.tile([1, 256], F32, tag="isb", name="isb")
            nc.sync.dma_start(vsb[:], vrow[:])
            nc.sync.dma_start(isb[:], irow[:])
            nc.gpsimd.partition_broadcast(vb[:], vsb[:])
            nc.gpsimd.partition_broadcast(ib[:], isb[:])

            new2 = [pool.tile([128, 512], F32, tag=f"n2{t}", name=f"n2{t}") for t in range(2)]
            for mt in range(2):
                ps = psum.tile([128, 512], F32, tag="ps", name="ps")
                for kt in range(2):
                    nc.tensor.matmul(ps[:], st[kt][:, mt * 128:(mt + 1) * 128],
                                     xs[kt][:], start=(kt == 0), stop=(kt == 1))
                nc.vector.tensor_scalar(new2[mt][:], ps[:], rc[:, mt:mt + 1],
                                        None, ALU.mult)
            # allgather new2 -> window rows for L3 (device-chained)
            bG_in = dram.tile([256, 512], F32, name="bG_in")
            bG_out = dram.tile([8, 256, 512], F32, name="bG_out")
            for t in range(2):
                nc.sync.dma_start(bG_in[t * 128:(t + 1) * 128, :], new2[t][:])
            nc.gpsimd.collective_compute(
                "AllGather", ALU.bypass, replica_groups=RG,
                ins=[bG_in.opt()], outs=[bG_out.opt()])
            for rt in range(2):
                n2full = work.tile([128, 64, 64], F32, tag="n2full",
                                   name="n2full", bufs=1)
                n2fv = n2full[:].rearrange("p a b -> p (a b)")
                for k in range(NCORES):
                    nc.sync.dma_start(
                        n2fv[:, k * 512:(k + 1) * 512],
                        bG_out[k, rt * 128:(rt + 1) * 128, :])
                n2wt = work.tile([128, L3_W, 64], F32, tag="n2wt",
                                 name="n2wt", bufs=1)
                nc.gpsimd.ap_gather(n2wt[:], n2full[:], iw[:], channels=128,
                                    num_elems=64, d=64, num_idxs=L3_W)
                nc.sync.dma_start(
                    o_n2w[rt * 128:(rt + 1) * 128, :],
                    n2wt[:].rearrange("p a b -> p (a b)"))
            # transpose new2 -> n2T [4 x [128,256]]
            n2T = [pool.tile([128, 256], F32, tag=f"n2T{t}", name=f"n2T{t}") for t in range(4)]
            for ct in range(4):
                for rt in range(2):
                    pst = psum.tile([128, 128], F32, tag="tp", name="tp")
                    nc.tensor.transpose(
                        pst[:], new2[rt][:, ct * 128:(ct + 1) * 128], ident32[:])
                    nc.any.tensor_copy(n2T[ct][:, rt * 128:(rt + 1) * 128], pst[:])
            # masked = n2T*valid + inv ; ln
            ln2T = [pool.tile([128, 256], F32, tag=f"ln2T{t}", name=f"ln2T{t}") for t in range(4)]
            sx3P = psumS.tile([1, 256], F32, tag="sx3", name="sx3")
            for ct in range(4):
                msk = work.tile([128, 256], F32, tag="msk", name="msk")
                nc.vector.tensor_tensor(msk[:], n2T[ct][:], vb[:], ALU.mult)
                nc.vector.tensor_tensor(msk[:], msk[:], ib[:], ALU.add)
                nc.scalar.activation(ln2T[ct][:], msk[:], AF.Ln)
                prod = work.tile([128, 256], F32, tag="prod", name="prod")
                nc.vector.tensor_tensor(prod[:], n2T[ct][:], ln2T[ct][:], ALU.mult)
                nc.tensor.matmul(sx3P[:], ones32[:], prod[:],
                                 start=(ct == 0), stop=(ct == 3))
            sx3sb = work.tile([1, 256], F32, tag="sx3sb", name="sx3sb")
            nc.any.tensor_copy(sx3sb[:], sx3P[:])
            bS3_in = dram.tile([1, 256], F32, name="bS3_in")
            bS3_out = dram.tile([1, 256], F32, name="bS3_out")
            nc.sync.dma_start(bS3_in[:], sx3sb[:])
            nc.gpsimd.collective_compute(
                "AllReduce", ALU.add, replica_groups=RG,
                ins=[bS3_in.opt()], outs=[bS3_out.opt()])
            nc.sync.dma_start(o_kl3[256:257, :], bS3_out[:])
            bC3_in = dram.tile([256, 256], F32, name="bC3_in")
            bC3_out = dram.tile([256, 256], F32, name="bC3_out")
            for it in range(2):
                psC = psum.tile([128, 256], F32, tag="psC", name="psC")
                for kt in range(4):
                    nc.tensor.matmul(psC[:], n2T[kt][:, it * 128:(it + 1) * 128],
                                     ln2T[kt][:], start=(kt == 0), stop=(kt == 3))
                c3t = work.tile([128, 256], F32, tag="c3t", name="c3t")
                nc.any.tensor_copy(c3t[:], psC[:])
                nc.sync.dma_start(bC3_in[it * 128:(it + 1) * 128, :], c3t[:])
            nc.gpsimd.collective_compute(
                "AllReduce", ALU.add, replica_groups=RG,
                ins=[bC3_in.opt()], outs=[bC3_out.opt()])
            nc.sync.dma_start(o_kl3[0:256, :], bC3_out[:])

    nc.finalize()
    return nc, ["x2s", "sel2T", "icnt2", "vrow", "irow", "iwin"], \
        ["n2w", "klpack3"]


def _build_l3():
    nc, tile, mybir = _build_common()
    from concourse.masks import make_identity
    F32, F16 = mybir.dt.float32, mybir.dt.float16
    I16, U32 = mybir.dt.int16, mybir.dt.uint32
    ALU, AX, AF = mybir.AluOpType, mybir.AxisListType, mybir.ActivationFunctionType

    n2w = nc.dram_tensor("n2w", [256, L3_W * 64], F32, kind="ExternalInput")
    selT = nc.dram_tensor("sel3T", [256, 256], mybir.dt.uint8,
                          kind="ExternalInput")
    icnt = nc.dram_tensor("icnt3", [256, 1], F32, kind="ExternalInput")
    bias = nc.dram_tensor("biasv", [256, 1], F32, kind="ExternalInput")
    idxyc = nc.dram_tensor("idxyc", [128, 2], I16, kind="ExternalInput")
    idxyd = nc.dram_tensor("idxyd", [128, 2], I16, kind="ExternalInput")
    wyr = nc.dram_tensor("wy", [1, 32], F32, kind="ExternalInput")
    idxxc = nc.dram_tensor("idxxc", [128, 512], I16, kind="ExternalInput")
    idxxd = nc.dram_tensor("idxxd", [128, 512], I16, kind="ExternalInput")
    wxr = nc.dram_tensor("wx", [1, 256], F32, kind="ExternalInput")
    o_lab = nc.dram_tensor("lab", [8, 128, 64], F32, kind="ExternalOutput")
    RG = [list(range(NCORES))]

    W = L3_W * 64
    with tile.TileContext(nc) as tc:
        with tc.tile_pool(name="sb", bufs=1) as pool, \
             tc.tile_pool(name="work", bufs=2) as work, \
             tc.tile_pool(name="big", bufs=1) as big, \
             tc.tile_pool(name="psum", bufs=2, space="PSUM") as psum, \
             tc.tile_pool(name="dram", bufs=1, space="DRAM") as dram:
            ident32 = pool.tile([128, 128], F32, tag="id32", name="id32")
            make_identity(nc, ident32[:])
            nw = [pool.tile([128, W], F32, tag=f"nw{t}", name=f"nw{t}") for t in range(2)]
            st = [pool.tile([128, 256], F32, tag=f"st{t}", name=f"st{t}") for t in range(2)]
            stu = [pool.tile([128, 256], mybir.dt.uint8, tag=f"stu{t}",
                             name=f"stu{t}") for t in range(2)]
            for t in range(2):
                nc.sync.dma_start(nw[t][:], n2w[t * 128:(t + 1) * 128, :])
                nc.sync.dma_start(stu[t][:], selT[t * 128:(t + 1) * 128, :])
                nc.any.tensor_copy(st[t][:], stu[t][:])
            cnt = pool.tile([128, 2], F32, tag="cnt", name="cnt")
            nc.sync.dma_start(cnt[:], icnt[:].rearrange("(a p) b -> p (a b)", a=2))
            rc = pool.tile([128, 2], F32, tag="rc", name="rc")
            nc.vector.reciprocal(rc[:], cnt[:])
            bv = pool.tile([128, 2], F32, tag="bv", name="bv")
            nc.sync.dma_start(bv[:], bias[:].rearrange("(a p) b -> p (a b)", a=2))
            iyc = pool.tile([128, 2], I16, tag="iyc", name="iyc")
            iyd = pool.tile([128, 2], I16, tag="iyd", name="iyd")
            ixc = pool.tile([128, 512], I16, tag="ixc", name="ixc")
            ixd = pool.tile([128, 512], I16, tag="ixd", name="ixd")
            for t_, s_ in ((iyc, idxyc), (iyd, idxyd), (ixc, idxxc), (ixd, idxxd)):
                nc.sync.dma_start(t_[:], s_[:])
            wyt = pool.tile([128, 32], F32, tag="wyt", name="wyt")
            wxt = pool.tile([128, 256], F32, tag="wxt", name="wxt")
            wsb = work.tile([1, 32], F32, tag="wsb", name="wsb")
            nc.sync.dma_start(wsb[:], wyr[:])
            nc.gpsimd.partition_broadcast(wyt[:], wsb[:])
            wsb2 = work.tile([1, 256], F32, tag="wsb2", name="wsb2")
            nc.sync.dma_start(wsb2[:], wxr[:])
            nc.gpsimd.partition_broadcast(wxt[:], wsb2[:])

            up = [big.tile([128, 8192, 1], F32, tag=f"up{t}", name=f"up{t}")
                  for t in range(2)]
            for mt in range(2):
                n3 = work.tile([128, W], F32, tag="n3", name="n3")
                for half, (c0, c1) in enumerate(((0, 512), (512, W))):
                    ps = psum.tile([128, c1 - c0], F32, tag=f"ps{half}", name=f"ps{half}")
                    for kt in range(2):
                        nc.tensor.matmul(ps[:],
                                         st[kt][:, mt * 128:(mt + 1) * 128],
                                         nw[kt][:, c0:c1],
                                         start=(kt == 0), stop=(kt == 1))
                    nc.vector.tensor_scalar(n3[:, c0:c1], ps[:],
                                            rc[:, mt:mt + 1], None, ALU.mult)
                nc.vector.tensor_scalar(n3[:], n3[:], bv[:, mt:mt + 1], None,
                                        ALU.add)
                # y-interp via gather: [128,10,64] -> c,d [128,32,64]
                yc = work.tile([128, 32, 64], F32, tag="yc", name="yc")
                yd = work.tile([128, 32, 64], F32, tag="yd", name="yd")
                ydr = work.tile([128, 2048, 1], F32, tag="ydr", name="ydr")
                n3v = n3[:].rearrange("p (y x) -> p y x", y=L3_W)
                nc.gpsimd.ap_gather(yc[:], n3v, iyc[:], channels=128,
                                    num_elems=L3_W, d=64, num_idxs=32)
                nc.gpsimd.ap_gather(yd[:], n3v, iyd[:], channels=128,
                                    num_elems=L3_W, d=64, num_idxs=32)
                yc3 = yc[:]
                yd3 = yd[:]
                ydr3 = ydr[:].rearrange("p (y x) o -> p y (x o)", y=32)
                wy3 = wyt[:, :, None].broadcast_to([128, 32, 64])
                nc.vector.tensor_tensor(ydr3, yd3, yc3, ALU.subtract)
                nc.vector.tensor_tensor(ydr3, ydr3, wy3, ALU.mult)
                nc.vector.tensor_tensor(ydr3, ydr3, yc3, ALU.add)
                # x-interp via gather on [128, 2048, 1] -> [128, 8192]
                xc = big.tile([128, 8192, 1], F32, tag="xc", name="xc")
                xd = up[mt]
                nc.gpsimd.ap_gather(xc[:], ydr[:], ixc[:], channels=128,
                                    num_elems=2048, d=1, num_idxs=8192)
                nc.gpsimd.ap_gather(xd[:], ydr[:], ixd[:], channels=128,
                                    num_elems=2048, d=1, num_idxs=8192)
                xc3 = xc[:].rearrange("p (y j) o -> p y (j o)", y=32)
                xd3 = xd[:].rearrange("p (y j) o -> p y (j o)", y=32)
                wx3 = wxt[:, None, :].broadcast_to([128, 32, 256])
                nc.vector.tensor_tensor(xd3, xd3, xc3, ALU.subtract)
                nc.vector.tensor_tensor(xd3, xd3, wx3, ALU.mult)
                nc.vector.tensor_tensor(xd3, xd3, xc3, ALU.add)
            # transpose + argmax
            lab = pool.tile([128, 64], F32, tag="lab", name="lab")
            upf = [u[:].rearrange("p n o -> p (n o)") for u in up]
            for pt in range(64):
                sc = work.tile([128, 256], F32, tag="sc", name="sc")
                for mt in range(2):
                    pst = psum.tile([128, 128], F32, tag="tp", name="tp")
                    nc.tensor.transpose(
                        pst[:], upf[mt][:, pt * 128:(pt + 1) * 128], ident32[:])
                    nc.any.tensor_copy(sc[:, mt * 128:(mt + 1) * 128], pst[:])
                mx = work.tile([128, 8], F32, tag="mx", name="mx")
                nc.vector.max(mx[:], sc[:])
                mi = work.tile([128, 8], U32, tag="mi", name="mi")
                nc.vector.max_index(mi[:], mx[:], sc[:])
                nc.vector.tensor_copy(lab[:, pt:pt + 1], mi[:, 0:1])
            bL_in = dram.tile([128, 64], F32, name="bL_in")
            bL_out = dram.tile([8, 128, 64], F32, name="bL_out")
            nc.sync.dma_start(bL_in[:], lab[:])
            nc.gpsimd.collective_compute(
                "AllGather", ALU.bypass, replica_groups=RG,
                ins=[bL_in.opt()], outs=[bL_out.opt()])
            nc.sync.dma_start(o_lab[:], bL_out[:])

    nc.finalize()
    return nc, ["n2w", "sel3T", "icnt3", "biasv", "idxyc", "idxyd", "wy",
                "idxxc", "idxxd", "wx"], ["lab"]


def _emit_l23(nc, tile, mybir, tc, x2s, klp, iwin, idxyc, idxyd, wyr,
              idxxc, idxxd, wxr, o_pack):
    """Fused L2+greedy2+greedy3+L3: everything after L1 in ONE program.

    Greedy selection runs on-device in SCATTERED form: instead of compacting
    representative rows to the top (reference), rows stay at their anchor
    index and an active-mask a[] marks representatives. Compaction is order-
    preserving, so all downstream math is value-identical; the final argmax
    indices are remapped on the host via a cumsum LUT over a3.
    """
    from concourse.masks import make_identity
    F32, F16 = mybir.dt.float32, mybir.dt.float16
    I16, U8, U32 = mybir.dt.int16, mybir.dt.uint8, mybir.dt.uint32
    ALU, AX, AF = mybir.AluOpType, mybir.AxisListType, mybir.ActivationFunctionType
    RG = [list(range(NCORES))]
    W = L3_W * 64

    def klprep(tc, pool, work, psT, ident32, Crows, sxrow, tag):
        """Build U = (sx_i + sx_j - C - C^T < 1.8) as two [128,256] 0/1 tiles.
        (0.5*t < 0.9 <=> t < 1.8 exactly: *0.5 is a power-of-2 scale.)"""
        Ct = [pool.tile([128, 256], F32, tag=f"Ct{tag}{t}", name=f"Ct{tag}{t}")
              for t in range(2)]
        for bi in range(2):
            for bj in range(2):
                pst = psT.tile([128, 128], F32, tag="tpK", name=f"tpK{tag}")
                nc.tensor.transpose(
                    pst[:], Crows[bi][:, bj * 128:(bj + 1) * 128], ident32[:])
                nc.any.tensor_copy(Ct[bj][:, bi * 128:(bi + 1) * 128], pst[:])
        # sx row -> broadcast + column
        sxb = pool.tile([128, 256], F32, tag=f"sxb{tag}", name=f"sxb{tag}")
        nc.gpsimd.partition_broadcast(sxb[:], sxrow[:])
        zp = work.tile([128, 256], F32, tag="zpK", name=f"zpK{tag}")
        nc.gpsimd.memset(zp[:], 0.0)
        nc.vector.tensor_copy(zp[0:1, :], sxrow[:])
        sxcol = [pool.tile([128, 1], F32, tag=f"sxc{tag}{t}",
                           name=f"sxc{tag}{t}") for t in range(2)]
        for bj in range(2):
            pst = psT.tile([128, 128], F32, tag="tpK", name=f"tpK2{tag}")
            nc.tensor.transpose(
                pst[:], zp[:, bj * 128:(bj + 1) * 128], ident32[:])
            nc.any.tensor_copy(sxcol[bj][:], pst[:, 0:1])
        U = [pool.tile([128, 256], F32, tag=f"U{tag}{t}", name=f"U{tag}{t}")
             for t in range(2)]
        for rt in range(2):
            t_ = work.tile([128, 256], F32, tag="tK", name=f"tK{tag}")
            nc.vector.tensor_scalar(t_[:], sxb[:], sxcol[rt][:], None, ALU.add)
            nc.vector.tensor_tensor(t_[:], t_[:], Crows[rt][:], ALU.subtract)
            nc.vector.tensor_tensor(t_[:], t_[:], Ct[rt][:], ALU.subtract)
            nc.vector.tensor_scalar(U[rt][:], t_[:], float(THR2), None,
                                    ALU.is_lt)
        return U

    def greedy(tc, pool, work, ident32, U, validrow, apad, tag):
        """Sequential greedy: apad[0:1,:] <- active mask. U: 2x[128,256] 0/1
        (already column-masked by valid). validrow: [1,256] 0/1 tile AP."""
        matched = pool.tile([1, 256], F32, tag=f"mt{tag}", name=f"mt{tag}")
        nc.gpsimd.memset(matched[:], 0.0)
        nc.gpsimd.memset(apad[:], 0.0)
        with tc.tile_pool(name=f"psE{tag}", bufs=4, space="PSUM") as psE:
            for i in range(256):
                ps = psE.tile([1, 256], F32, tag="ext", name=f"ext{tag}")
                nc.tensor.matmul(ps[:], ident32[:, i % 128:i % 128 + 1],
                                 U[i // 128][:], start=True, stop=True)
                # a_i = valid[i] - matched[i]  (matched <= valid always)
                nc.vector.tensor_tensor(
                    apad[0:1, i:i + 1], validrow[0:1, i:i + 1],
                    matched[0:1, i:i + 1], ALU.subtract)
                srow = work.tile([1, 256], F32, tag=f"sr{tag}",
                                 name=f"sr{tag}")
                nc.vector.tensor_scalar(srow[:], ps[0:1, :],
                                        apad[0:1, i:i + 1], None, ALU.mult)
                nc.vector.tensor_tensor(matched[0:1, :], matched[0:1, :],
                                        srow[:], ALU.max)

    if True:
        with tc.tile_pool(name="sb", bufs=1) as pool, \
             tc.tile_pool(name="work", bufs=2) as work, \
             tc.tile_pool(name="big", bufs=1) as big, \
             tc.tile_pool(name="dram", bufs=1, space="DRAM") as dram:
            ident32 = pool.tile([128, 128], F32, tag="id32", name="id32")
            make_identity(nc, ident32[:])
            ones32 = pool.tile([128, 1], F32, tag="o32", name="o32")
            nc.gpsimd.memset(ones32[:], 1.0)
            onesrow = pool.tile([1, 256], F32, tag="or", name="or")
            nc.gpsimd.memset(onesrow[:], 1.0)
            iw = pool.tile([128, 1], I16, tag="iw", name="iw")
            nc.sync.dma_start(iw[:], iwin[:])
            xs = [pool.tile([128, 512], F32, tag=f"xs{t}", name=f"xs{t}")
                  for t in range(2)]
            for t in range(2):
                nc.sync.dma_start(xs[t][:], x2s[t * 128:(t + 1) * 128, :])
            C1 = [work.tile([128, 256], F32, tag=f"C1_{t}", name=f"C1_{t}",
                            bufs=1) for t in range(2)]
            for t in range(2):
                nc.sync.dma_start(C1[t][:], klp[t * 128:(t + 1) * 128, :])
            sxr = work.tile([1, 256], F32, tag="sxr", name="sxr", bufs=1)
            nc.sync.dma_start(sxr[:], klp[256:257, :])

            # ---- greedy 2 (valid == ones) ----
            a2pad = pool.tile([128, 256], F32, tag="a2p", name="a2p")
            with tc.tile_pool(name="psT", bufs=2, space="PSUM") as psT:
                U2 = klprep(tc, pool, work, psT, ident32, C1, sxr, "2")
            greedy(tc, pool, work, ident32, U2, onesrow, a2pad, "2")

            a2bc = pool.tile([128, 256], F32, tag="a2bc", name="a2bc")
            nc.gpsimd.partition_broadcast(a2bc[:], a2pad[0:1, :])
            ib2 = pool.tile([128, 256], F32, tag="ib2", name="ib2")
            nc.vector.tensor_scalar(ib2[:], a2bc[:], -1.0, 1.0, ALU.mult,
                                    ALU.add)
            # selT2 = U2 * a2[free]; new2 = selT2^T @ X2 / cnt
            selT2 = [pool.tile([128, 256], F32, tag=f"sT2_{t}",
                               name=f"sT2_{t}") for t in range(2)]
            for t in range(2):
                nc.vector.tensor_tensor(selT2[t][:], U2[t][:], a2bc[:],
                                        ALU.mult)
            new2 = [pool.tile([128, 512], F32, tag=f"n2{t}", name=f"n2{t}")
                    for t in range(2)]
            rc2 = [pool.tile([128, 1], F32, tag=f"rc2{t}", name=f"rc2{t}")
                   for t in range(2)]
            with tc.tile_pool(name="psB", bufs=2, space="PSUM") as psum, \
                 tc.tile_pool(name="psS", bufs=1, space="PSUM") as psumS:
                for mt in range(2):
                    cc = psumS.tile([128, 1], F32, tag="cc", name="cc")
                    for kt in range(2):
                        nc.tensor.matmul(
                            cc[:], selT2[kt][:, mt * 128:(mt + 1) * 128],
                            ones32[:], start=(kt == 0), stop=(kt == 1))
                    nc.vector.tensor_scalar(rc2[mt][:], cc[:], 1.0, None,
                                            ALU.max)
                    nc.vector.reciprocal(rc2[mt][:], rc2[mt][:])
                for mt in range(2):
                    ps = psum.tile([128, 512], F32, tag="psN", name="psN")
                    for kt in range(2):
                        nc.tensor.matmul(
                            ps[:], selT2[kt][:, mt * 128:(mt + 1) * 128],
                            xs[kt][:], start=(kt == 0), stop=(kt == 1))
                    nc.vector.tensor_scalar(new2[mt][:], ps[:], rc2[mt][:],
                                            None, ALU.mult)

                # allgather new2 -> window rows for L3 part
                bG_in = dram.tile([256, 512], F32, name="bG_in")
                bG_out = dram.tile([8, 256, 512], F32, name="bG_out")
                for t in range(2):
                    nc.sync.dma_start(bG_in[t * 128:(t + 1) * 128, :],
                                      new2[t][:])
                nc.gpsimd.collective_compute(
                    "AllGather", ALU.bypass, replica_groups=RG,
                    ins=[bG_in.opt()], outs=[bG_out.opt()])
                nw = [pool.tile([128, W], F32, tag=f"nw{t}", name=f"nw{t}")
                      for t in range(2)]
                for rt in range(2):
                    n2full = big.tile([128, 64, 64], F32, tag="bigsc",
                                      name="n2full")
                    n2fv = n2full[:].rearrange("p a b -> p (a b)")
                    for k in range(NCORES):
                        nc.sync.dma_start(
                            n2fv[:, k * 512:(k + 1) * 512],
                            bG_out[k, rt * 128:(rt + 1) * 128, :])
                    n2wt = nw[rt][:].rearrange("p (a b) -> p a b", a=L3_W)
                    nc.gpsimd.ap_gather(n2wt, n2full[:], iw[:], channels=128,
                                        num_elems=64, d=64, num_idxs=L3_W)

                # n2T, masked ln, sx3/C3 partials -> single [257,256] AllReduce
                n2T = [pool.tile([128, 256], F32, tag=f"n2T{t}",
                                 name=f"n2T{t}") for t in range(4)]
                for ct in range(4):
                    for rt in range(2):
                        pst = psum.tile([128, 128], F32, tag="tp", name="tp")
                        nc.tensor.transpose(
                            pst[:], new2[rt][:, ct * 128:(ct + 1) * 128],
                            ident32[:])
                        nc.any.tensor_copy(
                            n2T[ct][:, rt * 128:(rt + 1) * 128], pst[:])
                ln2T = [work.tile([128, 256], F32, tag=f"ln2T{t}",
                                  name=f"ln2T{t}", bufs=1) for t in range(4)]
                sx3P = psumS.tile([1, 256], F32, tag="sx3", name="sx3")
                for ct in range(4):
                    msk = work.tile([128, 256], F32, tag="msk", name="msk")
                    nc.vector.tensor_tensor(msk[:], n2T[ct][:], a2bc[:],
                                            ALU.mult)
                    nc.vector.tensor_tensor(msk[:], msk[:], ib2[:], ALU.add)
                    nc.scalar.activation(ln2T[ct][:], msk[:], AF.Ln)
                    prod = work.tile([128, 256], F32, tag="prod", name="prod")
                    nc.vector.tensor_tensor(prod[:], n2T[ct][:], ln2T[ct][:],
                                            ALU.mult)
                    nc.tensor.matmul(sx3P[:], ones32[:], prod[:],
                                     start=(ct == 0), stop=(ct == 3))
                bK_in = dram.tile([257, 256], F32, name="bK_in")
                bK_out = dram.tile([257, 256], F32, name="bK_out")
                sx3sb = work.tile([1, 256], F32, tag="sx3sb", name="sx3sb")
                nc.any.tensor_copy(sx3sb[:], sx3P[:])
                nc.sync.dma_start(bK_in[256:257, :], sx3sb[:])
                for it in range(2):
                    psC = psum.tile([128, 256], F32, tag="psC", name="psC")
                    for kt in range(4):
                        nc.tensor.matmul(
                            psC[:], n2T[kt][:, it * 128:(it + 1) * 128],
                            ln2T[kt][:], start=(kt == 0), stop=(kt == 3))
                    c3t = work.tile([128, 256], F32, tag="c3t", name="c3t")
                    nc.any.tensor_copy(c3t[:], psC[:])
                    nc.sync.dma_start(bK_in[it * 128:(it + 1) * 128, :],
                                      c3t[:])
                nc.gpsimd.collective_compute(
                    "AllReduce", ALU.add, replica_groups=RG,
                    ins=[bK_in.opt()], outs=[bK_out.opt()])

            # ---- greedy 3 (valid == a2) ----
            C3 = [work.tile([128, 256], F32, tag=f"C3_{t}", name=f"C3_{t}",
                            bufs=1) for t in range(2)]
            for t in range(2):
                nc.sync.dma_start(C3[t][:], bK_out[t * 128:(t + 1) * 128, :])
            sx3r = work.tile([1, 256], F32, tag="sx3r", name="sx3r", bufs=1)
            nc.sync.dma_start(sx3r[:], bK_out[256:257, :])
            a3pad = pool.tile([128, 256], F32, tag="a3p", name="a3p")
            with tc.tile_pool(name="psT3", bufs=2, space="PSUM") as psT3:
                U3 = klprep(tc, pool, work, psT3, ident32, C3, sx3r, "3")
                for t in range(2):
                    nc.vector.tensor_tensor(U3[t][:], U3[t][:], a2bc[:],
                                            ALU.mult)
            greedy(tc, pool, work, ident32, U3, a2pad[0:1, :], a3pad, "3")

            # a3 columns, sel3T = transpose(U3 * a3col), cnt3, bias
            a3col = [pool.tile([128, 1], F32, tag=f"a3c{t}", name=f"a3c{t}")
                     for t in range(2)]
            sel3T = [pool.tile([128, 256], F32, tag=f"sT3_{t}",
                               name=f"sT3_{t}") for t in range(2)]
            rc3 = [pool.tile([128, 1], F32, tag=f"rc3{t}", name=f"rc3{t}")
                   for t in range(2)]
            bv = [pool.tile([128, 1], F32, tag=f"bv{t}", name=f"bv{t}")
                  for t in range(2)]
            up = [big.tile([128, 8192, 1], F32, tag=f"up{t}", name=f"up{t}")
                  for t in range(2)]
            with tc.tile_pool(name="psD", bufs=2, space="PSUM") as psum, \
                 tc.tile_pool(name="psS3", bufs=1, space="PSUM") as psumS:
                for bj in range(2):
                    pst = psum.tile([128, 128], F32, tag="tpD", name="tp3")
                    nc.tensor.transpose(
                        pst[:], a3pad[:, bj * 128:(bj + 1) * 128], ident32[:])
                    nc.any.tensor_copy(a3col[bj][:], pst[:, 0:1])
                S3 = [work.tile([128, 256], F32, tag=f"S3_{t}",
                                name=f"S3_{t}", bufs=1) for t in range(2)]
                for t in range(2):
                    nc.vector.tensor_scalar(S3[t][:], U3[t][:], a3col[t][:],
                                            None, ALU.mult)
                for bi in range(2):
                    for bj in range(2):
                        pst = psum.tile([128, 128], F32, tag="tpD",
                                        name="tp3b")
                        nc.tensor.transpose(
                            pst[:], S3[bi][:, bj * 128:(bj + 1) * 128],
                            ident32[:])
                        nc.any.tensor_copy(
                            sel3T[bj][:, bi * 128:(bi + 1) * 128], pst[:])
                for mt in range(2):
                    cc = psumS.tile([128, 1], F32, tag="cc3", name="cc3")
                    for kt in range(2):
                        nc.tensor.matmul(
                            cc[:], sel3T[kt][:, mt * 128:(mt + 1) * 128],
                            ones32[:], start=(kt == 0), stop=(kt == 1))
                    nc.vector.tensor_scalar(rc3[mt][:], cc[:], 1.0, None,
                                            ALU.max)
                    nc.vector.reciprocal(rc3[mt][:], rc3[mt][:])
                    nc.vector.tensor_scalar(bv[mt][:], a3col[mt][:], -1.0,
                                            float(-NEG_BIG), ALU.add,
                                            ALU.mult)

                # ---- L3: new3 window, upsample, argmax ----
                wyt = pool.tile([128, 32], F32, tag="wyt", name="wyt")
                wxt = pool.tile([128, 256], F32, tag="wxt", name="wxt")
                wsb = work.tile([1, 32], F32, tag="wsb", name="wsb")
                nc.sync.dma_start(wsb[:], wyr[:])
                nc.gpsimd.partition_broadcast(wyt[:], wsb[:])
                wsb2 = work.tile([1, 256], F32, tag="wsb2", name="wsb2")
                nc.sync.dma_start(wsb2[:], wxr[:])
                nc.gpsimd.partition_broadcast(wxt[:], wsb2[:])
                iyc = pool.tile([128, 2], I16, tag="iyc", name="iyc")
                iyd = pool.tile([128, 2], I16, tag="iyd", name="iyd")
                ixc = pool.tile([128, 512], I16, tag="ixc", name="ixc")
                ixd = pool.tile([128, 512], I16, tag="ixd", name="ixd")
                for t_, s_ in ((iyc, idxyc), (iyd, idxyd), (ixc, idxxc),
                               (ixd, idxxd)):
                    nc.sync.dma_start(t_[:], s_[:])

                for mt in range(2):
                    n3 = work.tile([128, W], F32, tag="n3", name="n3",
                                   bufs=1)
                    for half, (c0, c1) in enumerate(((0, 512), (512, W))):
                        ps = psum.tile([128, 512], F32, tag="psH",
                                       name=f"ps{half}")
                        psv = ps[:, 0:c1 - c0]
                        for kt in range(2):
                            nc.tensor.matmul(
                                psv, sel3T[kt][:, mt * 128:(mt + 1) * 128],
                                nw[kt][:, c0:c1],
                                start=(kt == 0), stop=(kt == 1))
                        nc.vector.tensor_scalar(n3[:, c0:c1], psv,
                                                rc3[mt][:], None, ALU.mult)
                    nc.vector.tensor_scalar(n3[:], n3[:], bv[mt][:], None,
                                            ALU.add)
                    yc = work.tile([128, 32, 64], F32, tag="yc", name="yc",
                                   bufs=1)
                    yd = work.tile([128, 32, 64], F32, tag="yd", name="yd",
                                   bufs=1)
                    ydr = work.tile([128, 2048, 1], F32, tag="ydr",
                                    name="ydr", bufs=1)
                    n3v = n3[:].rearrange("p (y x) -> p y x", y=L3_W)
                    nc.gpsimd.ap_gather(yc[:], n3v, iyc[:], channels=128,
                                        num_elems=L3_W, d=64, num_idxs=32)
                    nc.gpsimd.ap_gather(yd[:], n3v, iyd[:], channels=128,
                                        num_elems=L3_W, d=64, num_idxs=32)
                    ydr3 = ydr[:].rearrange("p (y x) o -> p y (x o)", y=32)
                    wy3 = wyt[:, :, None].broadcast_to([128, 32, 64])
                    nc.vector.tensor_tensor(ydr3, yd[:], yc[:], ALU.subtract)
                    nc.vector.tensor_tensor(ydr3, ydr3, wy3, ALU.mult)
                    nc.vector.tensor_tensor(ydr3, ydr3, yc[:], ALU.add)
                    xc = big.tile([128, 8192, 1], F32, tag="bigsc",
                                  name="xc")
                    xd = up[mt]
                    nc.gpsimd.ap_gather(xc[:], ydr[:], ixc[:], channels=128,
                                        num_elems=2048, d=1, num_idxs=8192)
                    nc.gpsimd.ap_gather(xd[:], ydr[:], ixd[:], channels=128,
                                        num_elems=2048, d=1, num_idxs=8192)
                    xc3 = xc[:].rearrange("p (y j) o -> p y (j o)", y=32)
                    xd3 = xd[:].rearrange("p (y j) o -> p y (j o)", y=32)
                    wx3 = wxt[:, None, :].broadcast_to([128, 32, 256])
                    nc.vector.tensor_tensor(xd3, xd3, xc3, ALU.subtract)
                    nc.vector.tensor_tensor(xd3, xd3, wx3, ALU.mult)
                    nc.vector.tensor_tensor(xd3, xd3, xc3, ALU.add)
                # transpose + argmax (labels as u8 scattered anchor idx)
                pack = pool.tile([128, 66], U8, tag="pack", name="pack")
                upf = [u[:].rearrange("p n o -> p (n o)") for u in up]
                for pt in range(64):
                    sc = work.tile([128, 256], F32, tag="sc", name="sc")
                    for mt in range(2):
                        pst = psum.tile([128, 128], F32, tag="tpD", name="tpA")
                        nc.tensor.transpose(
                            pst[:], upf[mt][:, pt * 128:(pt + 1) * 128],
                            ident32[:])
                        nc.any.tensor_copy(sc[:, mt * 128:(mt + 1) * 128],
                                           pst[:])
                    mx = work.tile([128, 8], F32, tag="mx", name="mx")
                    nc.vector.max(mx[:], sc[:])
                    mi = work.tile([128, 8], U32, tag="mi", name="mi")
                    nc.vector.max_index(mi[:], mx[:], sc[:])
                    nc.vector.tensor_copy(pack[:, pt:pt + 1], mi[:, 0:1])
                nc.vector.tensor_copy(pack[:, 64:65], a3col[0][:])
                nc.vector.tensor_copy(pack[:, 65:66], a3col[1][:])
                # per-core output; the host gathers the 8 shards in one
                # parallel device_get (cheaper than AllGather + 8x payload)
                nc.sync.dma_start(o_pack[:], pack[:])


def _build_l23():
    nc, tile, mybir = _build_common()
    F32, I16, U8 = mybir.dt.float32, mybir.dt.int16, mybir.dt.uint8
    x2s = nc.dram_tensor("x2s", [256, 512], F32, kind="ExternalInput")
    klp = nc.dram_tensor("klp", [257, 256], F32, kind="ExternalInput")
    iwin = nc.dram_tensor("iwin", [128, 1], I16, kind="ExternalInput")
    idxyc = nc.dram_tensor("idxyc", [128, 2], I16, kind="ExternalInput")
    idxyd = nc.dram_tensor("idxyd", [128, 2], I16, kind="ExternalInput")
    wyr = nc.dram_tensor("wy", [1, 32], F32, kind="ExternalInput")
    idxxc = nc.dram_tensor("idxxc", [128, 512], I16, kind="ExternalInput")
    idxxd = nc.dram_tensor("idxxd", [128, 512], I16, kind="ExternalInput")
    wxr = nc.dram_tensor("wx", [1, 256], F32, kind="ExternalInput")
    o_pack = nc.dram_tensor("opack", [128, 66], U8, kind="ExternalOutput")
    with tile.TileContext(nc) as tc:
        _emit_l23(nc, tile, mybir, tc, x2s, klp, iwin, idxyc, idxyd, wyr,
                  idxxc, idxxd, wxr, o_pack)
    nc.finalize()
    return nc, ["x2s", "klp", "iwin", "idxyc", "idxyd", "wy",
                "idxxc", "idxxd", "wx"], ["opack"]


def _build_l123():
    """Everything in ONE program: aggregation+merge0 (L1) then fused
    greedy/merge/upsample/argmax (L23), chained through internal DRAM."""
    nc, tile, mybir = _build_common()
    F32, I16, U8 = mybir.dt.float32, mybir.dt.int16, mybir.dt.uint8
    w64 = nc.dram_tensor("w64s", [8, 512, 4096], F32, kind="ExternalInput")
    w32 = nc.dram_tensor("w32s", [8, 256, 1024], F32, kind="ExternalInput")
    w16 = nc.dram_tensor("w16s", [8, 128, 256], F32, kind="ExternalInput")
    w8 = nc.dram_tensor("w8s", [8, 64, 64], F32, kind="ExternalInput")
    iwin = nc.dram_tensor("iwin", [128, 1], I16, kind="ExternalInput")
    idxyc = nc.dram_tensor("idxyc", [128, 2], I16, kind="ExternalInput")
    idxyd = nc.dram_tensor("idxyd", [128, 2], I16, kind="ExternalInput")
    wyr = nc.dram_tensor("wy", [1, 32], F32, kind="ExternalInput")
    idxxc = nc.dram_tensor("idxxc", [128, 512], I16, kind="ExternalInput")
    idxxd = nc.dram_tensor("idxxd", [128, 512], I16, kind="ExternalInput")
    wxr = nc.dram_tensor("wx", [1, 256], F32, kind="ExternalInput")
    o_pack = nc.dram_tensor("opack", [128, 66], U8, kind="ExternalOutput")
    with tile.TileContext(nc) as tc:
        with tc.tile_pool(name="xfer", bufs=1, space="DRAM") as xfer:
            x2d = xfer.tile([256, 512], F32, name="x2d")
            klpd = xfer.tile([257, 256], F32, name="klpd")
            _emit_l1(nc, tile, mybir, tc, w64, w32, w16, w8, x2d, klpd)
            _emit_l23(nc, tile, mybir, tc, x2d, klpd, iwin, idxyc, idxyd,
                      wyr, idxxc, idxxd, wxr, o_pack)
    nc.finalize()
    return nc, ["w64s", "w32s", "w16s", "w8s", "iwin", "idxyc", "idxyd",
                "wy", "idxxc", "idxxd", "wx"], ["opack"]


# ------------------------------------------------------------------- runner
class _Runner:
    """Cached shard_map-jitted executor for a finalized Bass program
    (modeled on bass2jax.run_bass_via_pjrt, but reusable across calls)."""

    def __init__(self, nc):
        import jax
        import jax.numpy as jnp
        from jax.sharding import Mesh, PartitionSpec, NamedSharding
        from jax.experimental.shard_map import shard_map
        from concourse import bass2jax as b2j
        from concourse import mybir
        b2j.install_neuronx_cc_hook()
        self.jax = jax
        self.np_outs = []
        in_names, out_names, out_avals, zero_outs = [], [], [], []
        partition_name = (nc.partition_id_tensor.name
                          if nc.partition_id_tensor else None)
        in_shapes = []
        for alloc in nc.m.functions[0].allocations:
            if not isinstance(alloc, mybir.MemoryLocationSet):
                continue
            name = alloc.memorylocations[0].name
            if alloc.kind == "ExternalInput":
                if name != partition_name:
                    in_names.append(name)
                    in_shapes.append((tuple(alloc.tensor_shape),
                                      mybir.dt.np(alloc.dtype)))
            elif alloc.kind == "ExternalOutput":
                shape = tuple(alloc.tensor_shape)
                dtype = mybir.dt.np(alloc.dtype)
                out_names.append(name)
                out_avals.append(jax.core.ShapedArray(shape, dtype))
                zero_outs.append(np.zeros(shape, dtype))
        self.in_names, self.out_names = in_names, out_names
        self.zero_outs = zero_outs
        n_params = len(in_names)
        bind_in_names = tuple(in_names + out_names +
                              ([partition_name] if partition_name else []))

        def _body(*args):
            operands = list(args)
            if partition_name is not None:
                operands.append(b2j.partition_id_tensor())
            outs = b2j._bass_exec_p.bind(
                *operands,
                out_avals=tuple(out_avals),
                in_names=bind_in_names,
                out_names=tuple(out_names),
                lowering_input_output_aliases=(),
                sim_require_finite=False,
                sim_require_nnan=False,
                nc=nc,
            )
            return tuple(outs)

        devices = jax.devices()[:NCORES]
        mesh = Mesh(np.asarray(devices), ("core",))
        n_outs = len(out_names)
        in_specs = (PartitionSpec("core"),) * (n_params + n_outs)
        out_specs = (PartitionSpec("core"),) * n_outs
        donate = tuple(range(n_params, n_params + n_outs))

        def _mk_jit():
            return jax.jit(
                shard_map(_body, mesh=mesh, in_specs=in_specs,
                          out_specs=out_specs, check_rep=False),
                donate_argnums=donate, keep_unused=True)

        self.out_avals = out_avals
        zsh = NamedSharding(mesh, PartitionSpec("core"))
        zspecs = [((NCORES * z.shape[0], *z.shape[1:]), z.dtype)
                  for z in zero_outs]
        # AOT-compile with the bass effect suppressed -> jax C++ fast-path
        # dispatch (~1ms less python overhead before the request hits the
        # wire). Fall back to the plain jit if unavailable.
        try:
            in_sds = [jax.ShapeDtypeStruct((NCORES * s[0], *s[1:]), d,
                                           sharding=zsh)
                      for (s, d) in in_shapes]
            z_sds = [jax.ShapeDtypeStruct(s, d, sharding=zsh)
                     for (s, d) in zspecs]
            self.fn = b2j.fast_dispatch_compile(
                lambda: _mk_jit().lower(*in_sds, *z_sds).compile())
        except Exception:
            self.fn = _mk_jit()
        # donated zero output buffers, created on-device (no H2D)
        self.zfn = jax.jit(
            lambda: tuple(jnp.zeros(s, d) for s, d in zspecs),
            out_shardings=tuple(zsh for _ in zspecs))
        self.in_sharding = zsh
        self._zcache = None

    def __call__(self, per_core_maps):
        concat_in = [np.concatenate([np.asarray(per_core_maps[c][nm])
                                     for c in range(NCORES)], axis=0)
                     for nm in self.in_names]
        return self.run_concat(concat_in)

    def run_raw(self, concat_in, zeros=None):
        """concat_in: list of [NCORES*s0, ...] arrays (np or device jax).
        Returns tuple of sharded jax output arrays. The donated output
        buffers come from a one-slot cache (stash_outputs recycles the
        previous call's fully-overwritten outputs), else a zeros jit."""
        concat_in = [x if hasattr(x, "addressable_shards")
                     else self.jax.device_put(np.ascontiguousarray(x),
                                              self.in_sharding)
                     for x in concat_in]
        if zeros is None:
            zeros, self._zcache = self._zcache, None
            if zeros is None:
                zeros = self.zfn()
        return self.fn(*concat_in, *zeros)

    def stash_outputs(self, raw):
        """Recycle output arrays as the next call's donated buffers (valid
        because every output byte is rewritten on device each run)."""
        self._zcache = tuple(raw)

    def run_concat(self, concat_in):
        out = self.run_raw(concat_in)
        res = []
        for c in range(NCORES):
            res.append({nm: np.asarray(out[i]).reshape(
                NCORES, *self.out_avals[i].shape)[c]
                for i, nm in enumerate(self.out_names)})
        return res


def _get_runner(name):
    if name not in _PROGS:
        build = {"l1": _build_l1, "l2": _build_l2, "l3": _build_l3,
                 "l23": _build_l23, "l123": _build_l123}[name]
        nc, ins, outs = build()
        _PROGS[name] = _Runner(nc)
    return _PROGS[name]


# --------------------------------------------------- on-device greedy (jax)
def _greedy_core_jax(klp, valid_row):
    """Replicates _klmat_host + _greedy decision-for-decision in f32.

    klp: [257,256] f32 (rows 0..255 = C, row 256 = sx), valid_row [1,256] f32.
    Returns compacted sel [256,256] f32, cnt [256] f32, newvalid [256] bool.
    """
    import jax.numpy as jnp
    from jax import lax
    C = klp[0:256]
    sx = klp[256]
    t = sx[:, None] + sx[None, :]
    t = t - C
    t = t - C.T
    kl = jnp.float32(0.5) * t
    valid = valid_row[0] > jnp.float32(0.5)
    kl = jnp.where(valid[None, :], kl, jnp.float32(np.inf))
    adj = (kl < jnp.float32(0.9)) & valid[None, :]

    def body(i, carry):
        matched, active = carry
        a = valid[i] & (~matched[i])
        matched = jnp.where(a, matched | adj[i], matched)
        active = active.at[i].set(a)
        return matched, active

    matched0 = jnp.zeros((256,), bool)
    active0 = jnp.zeros((256,), bool)
    _, active = lax.fori_loop(0, 256, body, (matched0, active0))
    ranks = jnp.cumsum(active.astype(jnp.int32)) - 1
    ocount = jnp.sum(active.astype(jnp.int32))
    rows = adj.astype(jnp.float32) * active[:, None].astype(jnp.float32)
    idx = jnp.where(active, ranks, 256)
    sel = jnp.zeros((257, 256), jnp.float32).at[idx].add(rows)[0:256]
    cnt = jnp.maximum(jnp.sum(sel, 1), jnp.float32(1.0))
    newvalid = jnp.arange(256) < ocount
    return sel, cnt, newvalid


def _get_greedy_jits():
    if "g2" not in _PROGS:
        import jax
        import jax.numpy as jnp
        from jax.sharding import Mesh, PartitionSpec
        from jax.experimental.shard_map import shard_map
        mesh = Mesh(np.asarray(jax.devices()[:NCORES]), ("core",))
        P = PartitionSpec

        def g2_body(klp):
            sel, cnt, newvalid = _greedy_core_jax(
                klp, jnp.ones((1, 256), jnp.float32))
            vrow = newvalid.astype(jnp.float32)[None, :]
            return (sel.T.astype(jnp.uint8), cnt[:, None],
                    vrow, jnp.float32(1.0) - vrow)

        def g3_body(klp, vrow2):
            sel, cnt, newvalid = _greedy_core_jax(klp, vrow2)
            biasv = jnp.where(newvalid, jnp.float32(0.0),
                              NEG_BIG)[:, None].astype(jnp.float32)
            return sel.T.astype(jnp.uint8), cnt[:, None], biasv

        _PROGS["g2"] = jax.jit(shard_map(
            g2_body, mesh=mesh, in_specs=(P("core"),),
            out_specs=(P("core"),) * 4, check_rep=False))
        _PROGS["g3"] = jax.jit(shard_map(
            g3_body, mesh=mesh, in_specs=(P("core"), P("core")),
            out_specs=(P("core"),) * 3, check_rep=False))
    return _PROGS["g2"], _PROGS["g3"]


_SMALL_DEV = {}


def _small_const_dev():
    """Per-call constant small inputs, staged on device once."""
    if not _SMALL_DEV:
        import jax
        from jax.sharding import Mesh, PartitionSpec, NamedSharding
        mesh = Mesh(np.asarray(jax.devices()[:NCORES]), ("core",))
        sh = NamedSharding(mesh, PartitionSpec("core"))
        _SMALL_DEV["iwin"] = jax.device_put(_IWIN_CAT, sh)
    return _SMALL_DEV


# ------------------------------------------------------------------- host math
def _greedy(klmat, valid):
    """Reference greedy loop via 256-bit ints. Returns sel bool [256,256], oc."""
    N = klmat.shape[0]
    adj = (klmat < np.float32(0.9)) & valid[None, :]
    rows = [int.from_bytes(np.packbits(adj[i], bitorder='little').tobytes(),
                           'little') for i in range(N)]
    vbits = int.from_bytes(np.packbits(valid, bitorder='little').tobytes(),
                           'little')
    matched = 0
    sel_rows = []
    for i in range(N):
        if (vbits >> i) & 1 and not (matched >> i) & 1:
            matched |= rows[i]
            sel_rows.append(rows[i])
    sel = np.zeros((N, N), bool)
    for o, r in enumerate(sel_rows):
        sel[o] = np.unpackbits(
            np.frombuffer(r.to_bytes(32, 'little'), np.uint8),
            bitorder='little')[:N]
    return sel, len(sel_rows)


def _klmat_host(sx, C):
    """0.5*(((sx_i+sx_j) - C) - C.T) in f32, matching the reference order."""
    t = (sx[:, None] + sx[None, :]).astype(np.float32)
    t = t - C
    t = t - C.T
    return (np.float32(0.5) * t).astype(np.float32)


def _prep_l1_inputs(w64, w32, w16, w8):
    cat64 = np.empty((64, 512, 4096), np.float32)
    cat32 = np.empty((64, 256, 1024), np.float32)
    cat16 = np.empty((64, 128, 256), np.float32)
    cat8 = np.empty((64, 64, 64), np.float32)
    for k in range(NCORES):
        cat64[8 * k:8 * k + 8] = w64[:, 512 * k:512 * k + 512, :]
        r32 = (8 * k) % 32 * 32
        cat32[8 * k:8 * k + 8] = w32[:, r32:r32 + 256, :]
        r16 = (8 * k) % 16 * 16
        cat16[8 * k:8 * k + 8] = w16[:, r16:r16 + 128, :]
        cat8[8 * k:8 * k + 8] = w8
    return [cat64, cat32, cat16, cat8]


def _segment_one(w64, w32, w16, w8, l1_dev_in=None):
    r = _get_runner("l123")
    wins = (l1_dev_in if l1_dev_in is not None
            else _prep_l1_inputs(w64, w32, w16, w8))
    per = {"w64s": wins[0], "w32s": wins[1], "w16s": wins[2], "w8s": wins[3],
           "iwin": _small_const_dev()["iwin"]}
    per.update(_l3_const_dev())
    raw = r.run_raw([per[nm] for nm in r.in_names])
    # single blocking host sync for the whole chain: fetch core 0's shard.
    # pack: [8,128,66] u8 = labels (scattered anchor idx, cols 0..63) +
    # the greedy-3 active mask a3 (cols 64/65).
    pack = np.asarray(raw[r.out_names.index("opack")]).reshape(8, 128, 66)
    r.stash_outputs(raw)   # recycle buffers: no zero-fill dispatch next call
    a3 = np.concatenate([pack[0][:, 64], pack[0][:, 65]]).astype(np.int32)
    lut = (np.cumsum(a3) - 1).astype(np.int32)   # scattered idx -> compact
    out = lut[pack[:, :, 0:64].transpose(0, 2, 1).reshape(-1)]
    return out.reshape(256, 256)


def kernel(**inputs):
    w64 = np.asarray(inputs["weight_64"], np.float32)
    w32 = np.asarray(inputs["weight_32"], np.float32)
    w16 = np.asarray(inputs["weight_16"], np.float32)
    w8 = np.asarray(inputs["weight_8"], np.float32)
    B = w64.shape[0]
    outs = [_segment_one(w64[b], w32[b], w16[b], w8[b]) for b in range(B)]
    return np.stack(outs).astype(np.int32)

